# revision 1
# baseline (speedup 1.0000x reference)
"""Trainium2 Bass kernel for a 4-layer LSTM autoencoder.

Contract: kernel(**inputs) takes the FULL fp32 inputs (B=65536) and returns
the full [B, T, D] fp32 reconstruction. Internally: pure data parallelism —
the batch is sharded across 8 NeuronCores; weights are replicated.

Device-side layout: everything is stored transposed, [feature=partitions,
batch=free]. Gates are computed as W_g @ x (+ W_hg @ h) with the batch
streaming through the PE array, so the recurrent state h never needs an
on-chip transpose. The host pre-transposes x and post-transposes the output.

Per layer-step (super-batch of 2048 columns):
  - per gate: 4 matmuls (input contribution, start=True) + 4 matmuls
    (recurrent, accumulate) into one 4-bank PSUM tile [128, 2048]
  - one ACT instruction per gate (sigmoid / tanh) PSUM -> SBUF bf16; biases
    are folded into the input matmul via an appended ones-row for layers
    whose input dim < 128 (enc0: 60, dec0: 64), and applied via the ACT
    per-partition bias operand for enc1/dec1 (input dim = 128).
  - DVE: c = f*c + i*g in fp32, h = o*tanh(c) in bf16
Encoder layers (and decoder layers) are pipelined with a 1-step skew so the
recurrence latency of one layer hides under the other layer's ACT work.
"""

import os
import sys
import time
from contextlib import ExitStack

import numpy as np

sys.path.insert(0, "/opt/trn_rl_repo")

import ml_dtypes  # noqa: E402

import concourse.bass as bass  # noqa: E402
import concourse.tile as tile  # noqa: E402
from concourse import bacc, mybir  # noqa: E402
from concourse.bass_utils import run_bass_kernel_spmd  # noqa: E402

F32 = mybir.dt.float32
BF16 = mybir.dt.bfloat16
SIG = mybir.ActivationFunctionType.Sigmoid
TANH = mybir.ActivationFunctionType.Tanh
IDENT = mybir.ActivationFunctionType.Identity
MULT = mybir.AluOpType.mult
ADD = mybir.AluOpType.add

B, T, D, H, L = 65536, 8, 60, 128, 64
N_CORES = 8
B_CORE = B // N_CORES        # 8192
SBW = 2048                   # super-batch width (columns in flight)
N_SB = B_CORE // SBW         # 4
CHUNK = 512                  # matmul moving-operand width (one PSUM bank)
N_CHUNKS = SBW // CHUNK      # 4

# layer descriptors: (name, input feature dim incl. ones-row, bias-in-ACT?)
LAYERS = {
    "enc0": dict(kin=D + 1, act_bias=False),
    "enc1": dict(kin=H, act_bias=True),
    "dec0": dict(kin=L + 1, act_bias=False),
    "dec1": dict(kin=H, act_bias=True),
}
GATE_FUNCS = [SIG, SIG, TANH, SIG]  # PyTorch gate order: i, f, g, o

_last_results = None  # set by kernel(); test harness reads exec_time_ns


def _build_kernel(trace: bool = False):
    nc = bacc.Bacc("TRN2", target_bir_lowering=False, debug=False,
                   num_devices=N_CORES)

    x_ext = nc.dram_tensor("x", [T, D + 1, B_CORE], BF16, kind="ExternalInput").ap()
    out_ext = nc.dram_tensor("out", [T, D, B_CORE], F32, kind="ExternalOutput").ap()

    w_in_ext, w_rec_ext, bias_ext = {}, {}, {}
    for name, cfg in LAYERS.items():
        w_in_ext[name] = nc.dram_tensor(
            f"{name}_w_in", [cfg["kin"], 4 * H], BF16, kind="ExternalInput").ap()
        w_rec_ext[name] = nc.dram_tensor(
            f"{name}_w_rec", [H, 4 * H], BF16, kind="ExternalInput").ap()
        if cfg["act_bias"]:
            bias_ext[name] = nc.dram_tensor(
                f"{name}_bias", [H, 4], F32, kind="ExternalInput").ap()
    w_lat_ext = nc.dram_tensor("w_lat", [H, L], BF16, kind="ExternalInput").ap()
    b_lat_ext = nc.dram_tensor("b_lat", [L, 1], F32, kind="ExternalInput").ap()
    w_out_ext = nc.dram_tensor("w_out", [H, D], BF16, kind="ExternalInput").ap()
    b_out_ext = nc.dram_tensor("b_out", [D, 1], F32, kind="ExternalInput").ap()

    with tile.TileContext(nc) as tc, ExitStack() as ctx:
        weights = ctx.enter_context(tc.tile_pool(name="weights", bufs=1))
        xpool = ctx.enter_context(tc.tile_pool(name="xpool", bufs=3))
        hpool = ctx.enter_context(tc.tile_pool(name="hpool", bufs=1))
        cpool = ctx.enter_context(tc.tile_pool(name="cpool", bufs=1))
        gpool = ctx.enter_context(tc.tile_pool(name="gpool", bufs=1))
        tpool = ctx.enter_context(tc.tile_pool(name="tpool", bufs=1))
        zpool = ctx.enter_context(tc.tile_pool(name="zpool", bufs=2))
        opool = ctx.enter_context(tc.tile_pool(name="opool", bufs=2))
        psA = ctx.enter_context(tc.tile_pool(name="psA", bufs=1, space="PSUM"))
        psB = ctx.enter_context(tc.tile_pool(name="psB", bufs=1, space="PSUM"))

        # ---- load weights once ----
        w_in, w_rec, w_bias = {}, {}, {}
        for name, cfg in LAYERS.items():
            w_in[name] = weights.tile([cfg["kin"], 4 * H], BF16, tag=f"wi_{name}", name=f"wi_{name}")
            nc.sync.dma_start(out=w_in[name], in_=w_in_ext[name][:, :])
            w_rec[name] = weights.tile([H, 4 * H], BF16, tag=f"wr_{name}", name=f"wr_{name}")
            nc.sync.dma_start(out=w_rec[name], in_=w_rec_ext[name][:, :])
            if cfg["act_bias"]:
                w_bias[name] = weights.tile([H, 4], F32, tag=f"wb_{name}", name=f"wb_{name}")
                nc.sync.dma_start(out=w_bias[name], in_=bias_ext[name][:, :])
        w_lat = weights.tile([H, L], BF16, tag="w_lat")
        nc.sync.dma_start(out=w_lat, in_=w_lat_ext[:, :])
        b_lat = weights.tile([L, 1], F32, tag="b_lat")
        nc.sync.dma_start(out=b_lat, in_=b_lat_ext[:, :])
        w_out = weights.tile([H, D], BF16, tag="w_out")
        nc.sync.dma_start(out=w_out, in_=w_out_ext[:, :])
        b_out = weights.tile([D, 1], F32, tag="b_out")
        nc.sync.dma_start(out=b_out, in_=b_out_ext[:, :])

        def lstm_step(name, t, rhs_in, h_prev, c_tile, ps_pool, ps_tag, hbufs):
            """Emit one LSTM step over SBW columns. Returns (h_new, c_tile)."""
            cfg = LAYERS[name]
            cls = "A" if name in ("enc0", "dec0") else "B"
            kin = cfg["kin"]
            gates = [None] * 4
            for g in range(4):
                if t == 0 and g == 1:
                    continue  # forget gate unused when c == 0
                gate = gpool.tile([H, SBW], BF16, tag=f"g{g}_{cls}", name=f"gate{g}_{name}_{t}")
                bias_arg = w_bias[name][:, g:g + 1] if cfg["act_bias"] else 0.0
                for half in range(2):
                    gps = ps_pool.tile([H, SBW // 2], F32, tag=ps_tag, bufs=2,
                                       name=f"gps_{name}_{t}_{g}_{half}")
                    for cc in range(N_CHUNKS // 2):
                        c = half * (N_CHUNKS // 2) + cc
                        s = bass.ts(c, CHUNK)
                        sh = bass.ts(cc, CHUNK)
                        nc.tensor.matmul(
                            gps[:, sh], w_in[name][:, bass.ts(g, H)], rhs_in[:kin, s],
                            start=True, stop=(t == 0))
                        if t > 0:
                            nc.tensor.matmul(
                                gps[:, sh], w_rec[name][:, bass.ts(g, H)], h_prev[:, s],
                                start=False, stop=True)
                    nc.scalar.activation(
                        out=gate[:, bass.ts(half, SBW // 2)], in_=gps,
                        func=GATE_FUNCS[g], bias=bias_arg)
                gates[g] = gate
            if t == 0:
                c_tile = cpool.tile([H, SBW], F32, tag=f"c_{name}", name=f"c_{name}_{t}")
                nc.vector.tensor_tensor(c_tile, gates[0], gates[2], MULT)
            else:
                t1 = tpool.tile([H, SBW], F32, tag="t1", name=f"t1_{name}_{t}")
                t2 = tpool.tile([H, SBW], F32, tag="t2", name=f"t2_{name}_{t}")
                nc.vector.tensor_tensor(t1, gates[0], gates[2], MULT)
                nc.vector.tensor_tensor(t2, gates[1], c_tile, MULT)
                nc.vector.tensor_tensor(c_tile, t1, t2, ADD)
            tc_t = tpool.tile([H, SBW], BF16, tag=f"tanhc_{cls}", name=f"tanhc_{name}_{t}")
            h_new = hpool.tile([H, SBW], BF16, tag=f"h_{name}", bufs=hbufs, name=f"h_{name}_{t}")
            for half in range(2):
                s = bass.ts(half, SBW // 2)
                nc.scalar.activation(out=tc_t[:, s], in_=c_tile[:, s], func=TANH)
                nc.vector.tensor_tensor(h_new[:, s], gates[3][:, s], tc_t[:, s], MULT)
            return h_new, c_tile

        for sb in range(N_SB):
            col0 = sb * SBW

            # ---------------- encoder ----------------
            ys = [None] * T
            h0 = c0 = h1 = c1 = None
            for slot in range(T + 1):
                if slot < T:
                    x_t = xpool.tile([D + 1, SBW], BF16, tag="x", name=f"x_{sb}_{slot}")
                    nc.sync.dma_start(
                        out=x_t, in_=x_ext[slot, :, col0:col0 + SBW])
                    h0, c0 = lstm_step("enc0", slot, x_t, h0, c0,
                                       psA, "gpsA", hbufs=3)
                    ys[slot] = h0
                if slot >= 1:
                    h1, c1 = lstm_step("enc1", slot - 1, ys[slot - 1], h1, c1,
                                       psB, "gpsB", hbufs=2)

            # ---------------- latent ----------------
            z_t = zpool.tile([L + 1, SBW], BF16, tag="z", name=f"z_{sb}")
            for half in range(2):
                gps = psB.tile([H, SBW // 2], F32, tag="gpsB", bufs=2,
                               name=f"lat_{sb}_{half}")
                for cc in range(N_CHUNKS // 2):
                    c = half * (N_CHUNKS // 2) + cc
                    nc.tensor.matmul(gps[:L, bass.ts(cc, CHUNK)], w_lat,
                                     h1[:, bass.ts(c, CHUNK)],
                                     start=True, stop=True)
                nc.scalar.activation(out=z_t[:L, bass.ts(half, SBW // 2)],
                                     in_=gps[:L, :], func=IDENT, bias=b_lat)
            nc.vector.memset(z_t[L:L + 1, :], 1.0)

            # ---------------- decoder ----------------
            d1 = [None] * T
            hd0 = cd0 = hd1 = cd1 = None
            for slot in range(T + 1):
                if slot < T:
                    hd0, cd0 = lstm_step("dec0", slot, z_t, hd0, cd0,
                                         psA, "gpsA", hbufs=3)
                    d1[slot] = hd0
                if slot >= 1:
                    td = slot - 1
                    hd1, cd1 = lstm_step("dec1", td, d1[td], hd1, cd1,
                                         psB, "gpsB", hbufs=2)
                    o_t = opool.tile([D, SBW], F32, tag="o", name=f"o_{td}")
                    for half in range(2):
                        gps = psB.tile([H, SBW // 2], F32, tag="gpsB", bufs=2,
                                       name=f"op_{td}_{half}")
                        for cc in range(N_CHUNKS // 2):
                            c = half * (N_CHUNKS // 2) + cc
                            nc.tensor.matmul(gps[:D, bass.ts(cc, CHUNK)], w_out,
                                             hd1[:, bass.ts(c, CHUNK)],
                                             start=True, stop=True)
                        nc.scalar.activation(out=o_t[:, bass.ts(half, SBW // 2)],
                                             in_=gps[:D, :], func=IDENT,
                                             bias=b_out)
                    nc.sync.dma_start(
                        out=out_ext[td, :, col0:col0 + SBW], in_=o_t)

    nc.finalize()
    return nc


def _prep_inputs(inputs):
    """Host-side: transpose/pack fp32 inputs into per-core device arrays."""
    x = inputs["x"]
    xt = np.ascontiguousarray(np.transpose(x, (1, 2, 0)))   # [T, D, B]
    ones = np.ones((T, 1, B), np.float32)
    xt = np.concatenate([xt, ones], axis=1).astype(ml_dtypes.bfloat16)

    common = {}
    for name in LAYERS:
        Wih = inputs[f"{name}_Wih"]
        Whh = inputs[f"{name}_Whh"]
        bsum = (inputs[f"{name}_bih"] + inputs[f"{name}_bhh"]).astype(np.float32)
        w_in = Wih.T.astype(np.float32)                      # [Din, 4H]
        if not LAYERS[name]["act_bias"]:
            w_in = np.concatenate([w_in, bsum[None, :]], axis=0)
        common[f"{name}_w_in"] = w_in.astype(ml_dtypes.bfloat16)
        common[f"{name}_w_rec"] = Whh.T.astype(ml_dtypes.bfloat16)
        if LAYERS[name]["act_bias"]:
            common[f"{name}_bias"] = np.ascontiguousarray(
                bsum.reshape(4, H).T)                        # [H, 4] fp32
    common["w_lat"] = inputs["W_lat"].T.astype(ml_dtypes.bfloat16)   # [H, L]
    common["b_lat"] = inputs["b_lat"].reshape(L, 1).astype(np.float32)
    common["w_out"] = inputs["W_out"].T.astype(ml_dtypes.bfloat16)   # [H, D]
    common["b_out"] = inputs["b_out"].reshape(D, 1).astype(np.float32)

    in_maps = []
    for core in range(N_CORES):
        m = dict(common)
        sl = slice(core * B_CORE, (core + 1) * B_CORE)
        m["x"] = np.ascontiguousarray(xt[:, :, sl])
        in_maps.append(m)
    return in_maps


def bench(inputs, reps: int = 8, reuse_nc=None):
    """Time repeated on-device executions (inputs device-resident, outputs
    left on device). Returns (best_seconds, all_times, outputs_of_first_run).
    """
    import jax
    from jax.sharding import Mesh, NamedSharding, PartitionSpec
    from jax.experimental.shard_map import shard_map
    from concourse import bass2jax
    from concourse.bass2jax import _bass_exec_p, partition_id_tensor

    bass2jax.install_neuronx_cc_hook()
    nc = reuse_nc if reuse_nc is not None else _build_kernel()
    in_maps = _prep_inputs(inputs)
    n_cores = N_CORES

    partition_name = nc.partition_id_tensor.name if nc.partition_id_tensor else None
    in_names, out_names, out_avals, zero_outs = [], [], [], []
    for alloc in nc.m.functions[0].allocations:
        if not isinstance(alloc, mybir.MemoryLocationSet):
            continue
        name = alloc.memorylocations[0].name
        if alloc.kind == "ExternalInput":
            if name != partition_name:
                in_names.append(name)
        elif alloc.kind == "ExternalOutput":
            out_names.append(name)
            out_avals.append(
                jax.core.ShapedArray(tuple(alloc.tensor_shape),
                                     mybir.dt.np(alloc.dtype)))
            zero_outs.append(
                np.zeros(tuple(alloc.tensor_shape), mybir.dt.np(alloc.dtype)))
    n_params = len(in_names)
    n_outs = len(out_names)
    all_in_names = in_names + out_names + ([partition_name] if partition_name else [])
    donate = tuple(range(n_params, n_params + n_outs))

    def _body(*args):
        operands = list(args)
        if partition_name is not None:
            operands.append(partition_id_tensor())
        return tuple(_bass_exec_p.bind(
            *operands, out_avals=tuple(out_avals), in_names=tuple(all_in_names),
            out_names=tuple(out_names), lowering_input_output_aliases=(),
            sim_require_finite=True, sim_require_nnan=True, nc=nc))

    devices = jax.devices()[:n_cores]
    mesh = Mesh(np.asarray(devices), ("core",))
    in_specs = (PartitionSpec("core"),) * (n_params + n_outs)
    out_specs = (PartitionSpec("core"),) * n_outs
    sharded = jax.jit(
        shard_map(_body, mesh=mesh, in_specs=in_specs, out_specs=out_specs,
                  check_rep=False),
        donate_argnums=donate, keep_unused=True)

    shard = NamedSharding(mesh, PartitionSpec("core"))
    concat_in = [
        jax.device_put(
            np.concatenate([np.asarray(in_maps[c][nm]) for c in range(n_cores)], 0),
            shard)
        for nm in in_names
    ]
    def fresh_zeros():
        return [jax.device_put(
                    np.zeros((n_cores * z.shape[0], *z.shape[1:]), z.dtype), shard)
                for z in zero_outs]

    # warm-up (compile)
    outs0 = sharded(*concat_in, *fresh_zeros())
    jax.block_until_ready(outs0)

    zero_sets = [fresh_zeros() for _ in range(reps)]
    jax.block_until_ready(zero_sets)
    times = []
    for r in range(reps):
        t0 = time.perf_counter()
        outs = sharded(*concat_in, *zero_sets[r])
        jax.block_until_ready(outs)
        times.append(time.perf_counter() - t0)
    return min(times), times, outs0


def kernel(**inputs) -> np.ndarray:
    global _last_results
    trace = bool(int(os.environ.get("BASS_LSTM_TRACE", "0")))
    nc = _build_kernel(trace)
    in_maps = _prep_inputs(inputs)
    res = run_bass_kernel_spmd(nc, in_maps, core_ids=list(range(N_CORES)),
                               trace=trace)
    _last_results = res
    outs = [res.results[c]["out"] for c in range(N_CORES)]   # [T, D, B_CORE]
    full = np.concatenate(outs, axis=2)                      # [T, D, B]
    return np.ascontiguousarray(np.transpose(full, (2, 0, 1)))  # [B, T, D]



# revision 19
# speedup vs baseline: 1.0885x; 1.0885x over previous
"""Trainium2 Bass kernel for a 4-layer LSTM autoencoder.

Contract: kernel(**inputs) takes the FULL fp32 inputs (B=65536) and returns
the full [B, T, D] fp32 reconstruction. Internally: pure data parallelism —
the batch is sharded across 8 NeuronCores; weights are replicated.

Device-side layout: everything is stored transposed, [feature=partitions,
batch=free]. Gates are computed as W_g @ x (+ W_hg @ h) with the batch
streaming through the PE array, so the recurrent state h never needs an
on-chip transpose. The host pre-transposes x and post-transposes the output.

The kernel is activation-engine bound (4 gate LUTs + tanh(c) per cell), so
the design keeps the ACT engine saturated with maximal-width instructions:

  - per layer-step (2048 columns): each gate gets a dedicated [128, 2048]
    fp32 PSUM tile (4 banks; 2 tiles ping-pong = all 8 banks). 4 input
    matmuls (start=True) then 4 recurrent matmuls fill it; ONE 2048-wide
    ACT (sigmoid/tanh) drains it to an fp16 SBUF gate tile. Biases are
    folded into the input matmul via an appended ones-row (enc0/dec0) or
    applied via the ACT per-partition bias operand (enc1/dec1).
  - DVE (all fp16, 2x perf mode): u = i*g, c = f*c + u, h = o*tanh(c).
  - latent/output-projection bias adds run on DVE (tensor_scalar_add with a
    per-partition bias AP) instead of ACT; the output projection is batched
    into an end phase over stored dec1 hiddens so it never steals PSUM or
    ACT slots from the recurrence.
Encoder layers (and decoder layers) are pipelined with a 1-step skew so the
recurrence latency of one layer hides under the other layer's ACT work.
"""

import os
import sys
import time
from contextlib import ExitStack

import numpy as np

sys.path.insert(0, "/opt/trn_rl_repo")

import concourse.bass as bass  # noqa: E402
import concourse.tile as tile  # noqa: E402
from concourse import bacc, mybir  # noqa: E402
from concourse.bass_utils import run_bass_kernel_spmd  # noqa: E402

F32 = mybir.dt.float32
F16 = mybir.dt.float16
SIG = mybir.ActivationFunctionType.Sigmoid
TANH = mybir.ActivationFunctionType.Tanh
MULT = mybir.AluOpType.mult
ADD = mybir.AluOpType.add

B, T, D, H, L = 65536, 8, 60, 128, 64
N_CORES = 8
B_CORE = B // N_CORES        # 8192
SBW = 2048                   # super-batch width (columns in flight)
N_SB = B_CORE // SBW         # 4
CHUNK = 512                  # matmul moving-operand width (one PSUM bank)
N_CHUNKS = SBW // CHUNK      # 4

# layer descriptors: (input feature dim incl. ones-row, bias-in-ACT?)
LAYERS = {
    "enc0": dict(kin=D + 1, act_bias=False),
    "enc1": dict(kin=H, act_bias=True),
    "dec0": dict(kin=L + 1, act_bias=False),
    "dec1": dict(kin=H, act_bias=True),
}
GATE_FUNCS = [SIG, SIG, TANH, SIG]  # PyTorch gate order: i, f, g, o

_last_results = None  # set by kernel(); test harness reads exec_time_ns


def _build_kernel(trace: bool = False):
    nc = bacc.Bacc("TRN2", target_bir_lowering=False, debug=False,
                   num_devices=N_CORES)

    x_ext = nc.dram_tensor("x", [T, D + 1, B_CORE], F16, kind="ExternalInput").ap()
    out_ext = nc.dram_tensor("out", [T, D, B_CORE], F32, kind="ExternalOutput").ap()

    w_in_ext, w_rec_ext, bias_ext = {}, {}, {}
    for name, cfg in LAYERS.items():
        w_in_ext[name] = nc.dram_tensor(
            f"{name}_w_in", [cfg["kin"], 4 * H], F16, kind="ExternalInput").ap()
        w_rec_ext[name] = nc.dram_tensor(
            f"{name}_w_rec", [H, 4 * H], F16, kind="ExternalInput").ap()
        if cfg["act_bias"]:
            bias_ext[name] = nc.dram_tensor(
                f"{name}_bias", [H, 4], F32, kind="ExternalInput").ap()
    w_lat_ext = nc.dram_tensor("w_lat", [H, L], F16, kind="ExternalInput").ap()
    b_lat_ext = nc.dram_tensor("b_lat", [L, 1], F32, kind="ExternalInput").ap()
    w_out_ext = nc.dram_tensor("w_out", [H, D], F16, kind="ExternalInput").ap()
    b_out_ext = nc.dram_tensor("b_out", [D, 1], F32, kind="ExternalInput").ap()

    with tile.TileContext(nc) as tc, ExitStack() as ctx:
        weights = ctx.enter_context(tc.tile_pool(name="weights", bufs=1))
        xpool = ctx.enter_context(tc.tile_pool(name="xpool", bufs=4))
        ypool = ctx.enter_context(tc.tile_pool(name="ypool", bufs=1))
        hpool = ctx.enter_context(tc.tile_pool(name="hpool", bufs=1))
        cpool = ctx.enter_context(tc.tile_pool(name="cpool", bufs=1))
        gpool = ctx.enter_context(tc.tile_pool(name="gpool", bufs=1))
        tpool = ctx.enter_context(tc.tile_pool(name="tpool", bufs=1))
        zpool = ctx.enter_context(tc.tile_pool(name="zpool", bufs=2))
        opool = ctx.enter_context(tc.tile_pool(name="opool", bufs=2))
        ps = ctx.enter_context(tc.tile_pool(name="ps", bufs=1, space="PSUM"))

        # ---- load weights once (encoder weights first; the rest is emitted
        # after the first slot so the HWDGE queue doesn't delay x[0]) ----
        w_in, w_rec, w_bias = {}, {}, {}

        def load_layer_weights(name):
            cfg = LAYERS[name]
            w_in[name] = weights.tile([cfg["kin"], 4 * H], F16,
                                      tag=f"wi_{name}", name=f"wi_{name}")
            nc.sync.dma_start(out=w_in[name], in_=w_in_ext[name][:, :])
            w_rec[name] = weights.tile([H, 4 * H], F16,
                                       tag=f"wr_{name}", name=f"wr_{name}")
            nc.sync.dma_start(out=w_rec[name], in_=w_rec_ext[name][:, :])
            if cfg["act_bias"]:
                w_bias[name] = weights.tile([H, 4], F32,
                                            tag=f"wb_{name}", name=f"wb_{name}")
                nc.sync.dma_start(out=w_bias[name], in_=bias_ext[name][:, :])

        load_layer_weights("enc0")

        def load_tail_weights():
            load_layer_weights("enc1")
            load_layer_weights("dec0")
            load_layer_weights("dec1")
            tw = {}
            tw["w_lat"] = weights.tile([H, L], F16, tag="w_lat", name="w_lat")
            nc.sync.dma_start(out=tw["w_lat"], in_=w_lat_ext[:, :])
            tw["b_lat"] = weights.tile([L, 1], F32, tag="b_lat", name="b_lat")
            nc.sync.dma_start(out=tw["b_lat"], in_=b_lat_ext[:, :])
            tw["w_out"] = weights.tile([H, D], F16, tag="w_out", name="w_out")
            nc.sync.dma_start(out=tw["w_out"], in_=w_out_ext[:, :])
            tw["b_out"] = weights.tile([D, 1], F32, tag="b_out", name="b_out")
            nc.sync.dma_start(out=tw["b_out"], in_=b_out_ext[:, :])
            return tw

        def step_gates(name, t, rhs_in, h_prev, c_tile):
            """Gate matmuls + gate ACTs + DVE cell update for one layer-step.

            Per gate: one [128, SBW] PSUM tile (input matmuls first — they
            only need rhs_in — then recurrent), drained by a single SBW-wide
            ACT into an fp16 gate tile. Returns (o_gate, c_tile). The tanh(c)
            + h = o*tanh(c) tail is emitted separately (step_tau_h) so the
            other layer's gate ACTs can fill the ACT pipeline while DVE
            finishes this layer's cell update.
            """
            cfg = LAYERS[name]
            cls = "A" if name in ("enc0", "dec0") else "B"
            kin = cfg["kin"]
            gates = [None] * 4
            for g in range(4):
                if t == 0 and g == 1:
                    continue  # forget gate unused when c == 0
                gps = ps.tile([H, SBW], F32, tag="gps", bufs=2,
                              name=f"gps_{name}_{t}_{g}")
                for c in range(N_CHUNKS):
                    nc.tensor.matmul(
                        gps[:, bass.ts(c, CHUNK)],
                        w_in[name][:, bass.ts(g, H)],
                        rhs_in[:kin, bass.ts(c, CHUNK)],
                        start=True, stop=(t == 0))
                if t > 0:
                    for c in range(N_CHUNKS):
                        nc.tensor.matmul(
                            gps[:, bass.ts(c, CHUNK)],
                            w_rec[name][:, bass.ts(g, H)],
                            h_prev[:, bass.ts(c, CHUNK)],
                            start=False, stop=True)
                gate = gpool.tile([H, SBW], F16, tag=f"g{g}_{cls}",
                                  name=f"gate{g}_{name}_{t}")
                bias_arg = w_bias[name][:, g:g + 1] if cfg["act_bias"] else 0.0
                nc.scalar.activation(out=gate, in_=gps, func=GATE_FUNCS[g],
                                     bias=bias_arg)
                gates[g] = gate
            if t == 0:
                c_tile = cpool.tile([H, SBW], F16, tag=f"c_{name}",
                                    name=f"c_{name}_{t}")
                nc.vector.tensor_tensor(c_tile, gates[0], gates[2], MULT)
            else:
                u = tpool.tile([H, SBW], F16, tag=f"u_{cls}",
                               name=f"u_{name}_{t}")
                nc.vector.tensor_tensor(u, gates[0], gates[2], MULT)
                nc.vector.tensor_tensor(c_tile, gates[1], c_tile, MULT)
                nc.vector.tensor_tensor(c_tile, c_tile, u, ADD)
            return gates[3], c_tile

        def emit_outproj(sb, col0, td, htile):
            ops_t = ps.tile([H, SBW], F32, tag="gps", bufs=2,
                            name=f"op_{sb}_{td}")
            for c in range(N_CHUNKS):
                nc.tensor.matmul(ops_t[:D, bass.ts(c, CHUNK)], w_out,
                                 htile[:, bass.ts(c, CHUNK)],
                                 start=True, stop=True)
            o_t = opool.tile([D, SBW], F32, tag="o", name=f"o_{sb}_{td}")
            nc.vector.tensor_scalar_add(o_t, ops_t[:D, :], b_out)
            nc.sync.dma_start(out=out_ext[td, :, col0:col0 + SBW], in_=o_t)

        def step_tau_h(name, t, o_gate, c_tile, h_tag, h_bufs):
            cls = "A" if name in ("enc0", "dec0") else "B"
            tau = tpool.tile([H, SBW], F16, tag=f"tau_{cls}",
                             name=f"tau_{name}_{t}")
            nc.scalar.activation(out=tau, in_=c_tile, func=TANH)
            h_new = hpool.tile([H, SBW], F16, tag=h_tag, bufs=h_bufs,
                               name=f"h_{name}_{t}")
            nc.vector.tensor_tensor(h_new, o_gate, tau, MULT)
            return h_new

        # Software-pipelined over super-batches: while sb k runs its decoder,
        # sb k+1 runs its encoder, so the ACT engine always has a second
        # stream of gate work during latent/out-projection/warm-up phases.
        xq = {}
        x_order = [(s, t) for s in range(N_SB) for t in range(T)]
        x_state = [0]

        def x_ensure(upto_idx):
            while x_state[0] <= min(upto_idx, len(x_order) - 1):
                sbi, ti = x_order[x_state[0]]
                xt = xpool.tile([D + 1, SBW], F16, tag="x",
                                name=f"x_{sbi}_{ti}")
                nc.sync.dma_start(
                    out=xt, in_=x_ext[ti, :, sbi * SBW:(sbi + 1) * SBW])
                xq[(sbi, ti)] = xt
                x_state[0] += 1

        def emit_latent(sb, h1_tile):
            z_t = zpool.tile([L + 1, SBW], F16, tag="z", name=f"z_{sb}")
            lat_ps = ps.tile([H, SBW], F32, tag="gps", bufs=2,
                             name=f"lat_{sb}")
            for c in range(N_CHUNKS):
                nc.tensor.matmul(lat_ps[:L, bass.ts(c, CHUNK)], w_lat,
                                 h1_tile[:, bass.ts(c, CHUNK)],
                                 start=True, stop=True)
            nc.vector.tensor_scalar_add(z_t[:L, :], lat_ps[:L, :], b_lat)
            nc.vector.memset(z_t[L:L + 1, :], 1.0)
            return z_t

        pend_h1 = None
        tw = None
        for phase in range(N_SB + 1):
            enc_sb = phase if phase < N_SB else None
            dec_sb = phase - 1 if phase >= 1 else None
            z_dec = None

            if enc_sb is not None:
                ys = [None] * T
                h0 = c0 = h1 = c1 = None
            if dec_sb is not None:
                dcol0 = dec_sb * SBW
                d1 = [None] * T
                hd0 = cd0 = hd1 = cd1 = None
                op_queue = []

            for slot in range(T + 1):
                # Interleave: each layer's tanh/h tail is emitted between the
                # other layers' gate blocks so every h is ready well before
                # the next slot's recurrent matmuls need it, and the ACT
                # engine never drains its queue. The (one-slot-delayed)
                # out-projection sits mid-slot so its DVE-paced PSUM-ring
                # retirement never gates a slot boundary.
                o0 = o1 = od0 = od1 = None
                if enc_sb is not None:
                    if slot < T:
                        x_ensure(enc_sb * T + slot + 2)
                        x_t = xq.pop((enc_sb, slot))
                        o0, c0 = step_gates("enc0", slot, x_t, h0, c0)
                    if slot >= 1:
                        o1, c1 = step_gates("enc1", slot - 1, ys[slot - 1],
                                            h1, c1)
                if tw is None:
                    tw = load_tail_weights()
                    w_lat, b_lat = tw["w_lat"], tw["b_lat"]
                    w_out, b_out = tw["w_out"], tw["b_out"]
                if o0 is not None:
                    h0 = step_tau_h("enc0", slot, o0, c0, "y0", 3)
                    ys[slot] = h0
                if dec_sb is not None and slot == 0:
                    # latent for this phase's decoder, emitted after the new
                    # encoder's first gates so those cover its latency
                    z_dec = emit_latent(dec_sb, pend_h1)
                if dec_sb is not None and slot < T:
                    od0, cd0 = step_gates("dec0", slot, z_dec, hd0, cd0)
                if o1 is not None:
                    h1 = step_tau_h("enc1", slot - 1, o1, c1, "h1", 2)
                if dec_sb is not None and op_queue and (
                        len(op_queue) > 1 or slot == T):
                    emit_outproj(dec_sb, dcol0, *op_queue.pop(0))
                if dec_sb is not None and slot >= 1:
                    od1, cd1 = step_gates("dec1", slot - 1, d1[slot - 1],
                                          hd1, cd1)
                if od0 is not None:
                    hd0 = step_tau_h("dec0", slot, od0, cd0, "y1", 3)
                    d1[slot] = hd0
                if od1 is not None:
                    hd1 = step_tau_h("dec1", slot - 1, od1, cd1, "d2", 3)
                    op_queue.append((slot - 1, hd1))

            if dec_sb is not None:
                for td, htile in op_queue:
                    emit_outproj(dec_sb, dcol0, td, htile)
            if enc_sb is not None:
                pend_h1 = h1

    nc.finalize()
    return nc


def _prep_inputs(inputs):
    """Host-side: transpose/pack fp32 inputs into per-core device arrays."""
    x = inputs["x"]
    xt = np.ascontiguousarray(np.transpose(x, (1, 2, 0)))   # [T, D, B]
    ones = np.ones((T, 1, B), np.float32)
    xt = np.concatenate([xt, ones], axis=1).astype(np.float16)

    common = {}
    for name in LAYERS:
        Wih = inputs[f"{name}_Wih"]
        Whh = inputs[f"{name}_Whh"]
        bsum = (inputs[f"{name}_bih"] + inputs[f"{name}_bhh"]).astype(np.float32)
        w_in = Wih.T.astype(np.float32)                      # [Din, 4H]
        if not LAYERS[name]["act_bias"]:
            w_in = np.concatenate([w_in, bsum[None, :]], axis=0)
        common[f"{name}_w_in"] = w_in.astype(np.float16)
        common[f"{name}_w_rec"] = Whh.T.astype(np.float16)
        if LAYERS[name]["act_bias"]:
            common[f"{name}_bias"] = np.ascontiguousarray(
                bsum.reshape(4, H).T)                        # [H, 4] fp32
    common["w_lat"] = inputs["W_lat"].T.astype(np.float16)   # [H, L]
    common["b_lat"] = inputs["b_lat"].reshape(L, 1).astype(np.float32)
    common["w_out"] = inputs["W_out"].T.astype(np.float16)   # [H, D]
    common["b_out"] = inputs["b_out"].reshape(D, 1).astype(np.float32)

    in_maps = []
    for core in range(N_CORES):
        m = dict(common)
        sl = slice(core * B_CORE, (core + 1) * B_CORE)
        m["x"] = np.ascontiguousarray(xt[:, :, sl])
        in_maps.append(m)
    return in_maps


def bench(inputs, reps: int = 8, reuse_nc=None):
    """Time repeated on-device executions (inputs device-resident, outputs
    left on device). Returns (best_seconds, all_times, outputs_of_first_run).
    """
    import jax
    from jax.sharding import Mesh, NamedSharding, PartitionSpec
    from jax.experimental.shard_map import shard_map
    from concourse import bass2jax
    from concourse.bass2jax import _bass_exec_p, partition_id_tensor

    bass2jax.install_neuronx_cc_hook()
    nc = reuse_nc if reuse_nc is not None else _build_kernel()
    in_maps = _prep_inputs(inputs)
    n_cores = N_CORES

    partition_name = nc.partition_id_tensor.name if nc.partition_id_tensor else None
    in_names, out_names, out_avals, zero_outs = [], [], [], []
    for alloc in nc.m.functions[0].allocations:
        if not isinstance(alloc, mybir.MemoryLocationSet):
            continue
        name = alloc.memorylocations[0].name
        if alloc.kind == "ExternalInput":
            if name != partition_name:
                in_names.append(name)
        elif alloc.kind == "ExternalOutput":
            out_names.append(name)
            out_avals.append(
                jax.core.ShapedArray(tuple(alloc.tensor_shape),
                                     mybir.dt.np(alloc.dtype)))
            zero_outs.append(
                np.zeros(tuple(alloc.tensor_shape), mybir.dt.np(alloc.dtype)))
    n_params = len(in_names)
    n_outs = len(out_names)
    all_in_names = in_names + out_names + ([partition_name] if partition_name else [])
    donate = tuple(range(n_params, n_params + n_outs))

    def _body(*args):
        operands = list(args)
        if partition_name is not None:
            operands.append(partition_id_tensor())
        return tuple(_bass_exec_p.bind(
            *operands, out_avals=tuple(out_avals), in_names=tuple(all_in_names),
            out_names=tuple(out_names), lowering_input_output_aliases=(),
            sim_require_finite=True, sim_require_nnan=True, nc=nc))

    devices = jax.devices()[:n_cores]
    mesh = Mesh(np.asarray(devices), ("core",))
    in_specs = (PartitionSpec("core"),) * (n_params + n_outs)
    out_specs = (PartitionSpec("core"),) * n_outs
    sharded = jax.jit(
        shard_map(_body, mesh=mesh, in_specs=in_specs, out_specs=out_specs,
                  check_rep=False),
        donate_argnums=donate, keep_unused=True)

    shard = NamedSharding(mesh, PartitionSpec("core"))
    concat_in = [
        jax.device_put(
            np.concatenate([np.asarray(in_maps[c][nm]) for c in range(n_cores)], 0),
            shard)
        for nm in in_names
    ]
    def fresh_zeros():
        return [jax.device_put(
                    np.zeros((n_cores * z.shape[0], *z.shape[1:]), z.dtype), shard)
                for z in zero_outs]

    # warm-up (compile)
    outs0 = sharded(*concat_in, *fresh_zeros())
    jax.block_until_ready(outs0)

    zero_sets = [fresh_zeros() for _ in range(reps)]
    jax.block_until_ready(zero_sets)
    times = []
    for r in range(reps):
        t0 = time.perf_counter()
        outs = sharded(*concat_in, *zero_sets[r])
        jax.block_until_ready(outs)
        times.append(time.perf_counter() - t0)
    return min(times), times, outs0


def kernel(**inputs) -> np.ndarray:
    global _last_results
    trace = bool(int(os.environ.get("BASS_LSTM_TRACE", "0")))
    nc = _build_kernel(trace)
    in_maps = _prep_inputs(inputs)
    res = run_bass_kernel_spmd(nc, in_maps, core_ids=list(range(N_CORES)),
                               trace=trace)
    _last_results = res
    outs = [res.results[c]["out"] for c in range(N_CORES)]   # [T, D, B_CORE]
    full = np.concatenate(outs, axis=2)                      # [T, D, B]
    return np.ascontiguousarray(np.transpose(full, (2, 0, 1)))  # [B, T, D]


# revision 24
# speedup vs baseline: 1.0889x; 1.0003x over previous
"""Trainium2 Bass kernel for a 4-layer LSTM autoencoder.

Contract: kernel(**inputs) takes the FULL fp32 inputs (B=65536) and returns
the full [B, T, D] fp32 reconstruction. Internally: pure data parallelism —
the batch is sharded across 8 NeuronCores; weights are replicated.

Device-side layout: everything is stored transposed, [feature=partitions,
batch=free]. Gates are computed as W_g @ x (+ W_hg @ h) with the batch
streaming through the PE array, so the recurrent state h never needs an
on-chip transpose. The host pre-transposes x and post-transposes the output.

The kernel is activation-engine bound (4 gate LUTs + tanh(c) per cell), so
the design keeps the ACT engine saturated with maximal-width instructions:

  - per layer-step (2048 columns): each gate gets a dedicated [128, 2048]
    fp32 PSUM tile (4 banks; 2 tiles ping-pong = all 8 banks). 4 input
    matmuls (start=True) then 4 recurrent matmuls fill it; ONE 2048-wide
    ACT (sigmoid/tanh) drains it to an fp16 SBUF gate tile. Biases are
    folded into the input matmul via an appended ones-row (enc0/dec0) or
    applied via the ACT per-partition bias operand (enc1/dec1).
  - DVE (all fp16, 2x perf mode): u = i*g, c = f*c + u, h = o*tanh(c).
  - latent/output-projection bias adds run on DVE (tensor_scalar_add with a
    per-partition bias AP) instead of ACT; the output projection is batched
    into an end phase over stored dec1 hiddens so it never steals PSUM or
    ACT slots from the recurrence.
Encoder layers (and decoder layers) are pipelined with a 1-step skew so the
recurrence latency of one layer hides under the other layer's ACT work.
"""

import os
import sys
import time
from contextlib import ExitStack

import numpy as np

sys.path.insert(0, "/opt/trn_rl_repo")

import concourse.bass as bass  # noqa: E402
import concourse.tile as tile  # noqa: E402
from concourse import bacc, mybir  # noqa: E402
from concourse.bass_utils import run_bass_kernel_spmd  # noqa: E402

F32 = mybir.dt.float32
F16 = mybir.dt.float16
SIG = mybir.ActivationFunctionType.Sigmoid
TANH = mybir.ActivationFunctionType.Tanh
MULT = mybir.AluOpType.mult
ADD = mybir.AluOpType.add
MIN = mybir.AluOpType.min
MAX = mybir.AluOpType.max
DIV = mybir.AluOpType.divide

B, T, D, H, L = 65536, 8, 60, 128, 64
N_CORES = 8
B_CORE = B // N_CORES        # 8192
SBW = 2048                   # super-batch width (columns in flight)
N_SB = B_CORE // SBW         # 4
CHUNK = 512                  # matmul moving-operand width (one PSUM bank)
N_CHUNKS = SBW // CHUNK      # 4

# layer descriptors: (input feature dim incl. ones-row, bias-in-ACT?)
LAYERS = {
    "enc0": dict(kin=D + 1, act_bias=False),
    "enc1": dict(kin=H, act_bias=True),
    "dec0": dict(kin=L + 1, act_bias=False),
    "dec1": dict(kin=H, act_bias=True),
}
GATE_FUNCS = [SIG, SIG, TANH, SIG]  # PyTorch gate order: i, f, g, o

_last_results = None  # set by kernel(); test harness reads exec_time_ns


def _build_kernel(trace: bool = False):
    nc = bacc.Bacc("TRN2", target_bir_lowering=False, debug=False,
                   num_devices=N_CORES)

    x_ext = nc.dram_tensor("x", [T, D + 1, B_CORE], F16, kind="ExternalInput").ap()
    out_ext = nc.dram_tensor("out", [T, D, B_CORE], F32, kind="ExternalOutput").ap()

    w_in_ext, w_rec_ext, bias_ext = {}, {}, {}
    for name, cfg in LAYERS.items():
        w_in_ext[name] = nc.dram_tensor(
            f"{name}_w_in", [cfg["kin"], 4 * H], F16, kind="ExternalInput").ap()
        w_rec_ext[name] = nc.dram_tensor(
            f"{name}_w_rec", [H, 4 * H], F16, kind="ExternalInput").ap()
        if cfg["act_bias"]:
            bias_ext[name] = nc.dram_tensor(
                f"{name}_bias", [H, 4], F32, kind="ExternalInput").ap()
    w_lat_ext = nc.dram_tensor("w_lat", [H, L], F16, kind="ExternalInput").ap()
    b_lat_ext = nc.dram_tensor("b_lat", [L, 1], F32, kind="ExternalInput").ap()
    w_out_ext = nc.dram_tensor("w_out", [H, D], F16, kind="ExternalInput").ap()
    b_out_ext = nc.dram_tensor("b_out", [D, 1], F32, kind="ExternalInput").ap()

    with tile.TileContext(nc) as tc, ExitStack() as ctx:
        weights = ctx.enter_context(tc.tile_pool(name="weights", bufs=1))
        xpool = ctx.enter_context(tc.tile_pool(name="xpool", bufs=4))
        ypool = ctx.enter_context(tc.tile_pool(name="ypool", bufs=1))
        hpool = ctx.enter_context(tc.tile_pool(name="hpool", bufs=1))
        cpool = ctx.enter_context(tc.tile_pool(name="cpool", bufs=1))
        gpool = ctx.enter_context(tc.tile_pool(name="gpool", bufs=1))
        tpool = ctx.enter_context(tc.tile_pool(name="tpool", bufs=1))
        zpool = ctx.enter_context(tc.tile_pool(name="zpool", bufs=2))
        opool = ctx.enter_context(tc.tile_pool(name="opool", bufs=2))
        ps = ctx.enter_context(tc.tile_pool(name="ps", bufs=1, space="PSUM"))

        # ---- load weights once (encoder weights first; the rest is emitted
        # after the first slot so the HWDGE queue doesn't delay x[0]) ----
        w_in, w_rec, w_bias = {}, {}, {}

        def load_layer_weights(name):
            cfg = LAYERS[name]
            w_in[name] = weights.tile([cfg["kin"], 4 * H], F16,
                                      tag=f"wi_{name}", name=f"wi_{name}")
            nc.sync.dma_start(out=w_in[name], in_=w_in_ext[name][:, :])
            w_rec[name] = weights.tile([H, 4 * H], F16,
                                       tag=f"wr_{name}", name=f"wr_{name}")
            nc.sync.dma_start(out=w_rec[name], in_=w_rec_ext[name][:, :])
            if cfg["act_bias"]:
                w_bias[name] = weights.tile([H, 4], F32,
                                            tag=f"wb_{name}", name=f"wb_{name}")
                nc.sync.dma_start(out=w_bias[name], in_=bias_ext[name][:, :])

        load_layer_weights("enc0")

        def load_tail_weights():
            load_layer_weights("enc1")
            load_layer_weights("dec0")
            load_layer_weights("dec1")
            tw = {}
            tw["w_lat"] = weights.tile([H, L], F16, tag="w_lat", name="w_lat")
            nc.sync.dma_start(out=tw["w_lat"], in_=w_lat_ext[:, :])
            tw["b_lat"] = weights.tile([L, 1], F32, tag="b_lat", name="b_lat")
            nc.sync.dma_start(out=tw["b_lat"], in_=b_lat_ext[:, :])
            tw["w_out"] = weights.tile([H, D], F16, tag="w_out", name="w_out")
            nc.sync.dma_start(out=tw["w_out"], in_=w_out_ext[:, :])
            tw["b_out"] = weights.tile([D, 1], F32, tag="b_out", name="b_out")
            nc.sync.dma_start(out=tw["b_out"], in_=b_out_ext[:, :])
            return tw

        def step_gates(name, t, rhs_in, h_prev, c_tile):
            """Gate matmuls + gate ACTs + DVE cell update for one layer-step.

            Per gate: one [128, SBW] PSUM tile (input matmuls first — they
            only need rhs_in — then recurrent), drained by a single SBW-wide
            ACT into an fp16 gate tile. Returns (o_gate, c_tile). The tanh(c)
            + h = o*tanh(c) tail is emitted separately (step_tau_h) so the
            other layer's gate ACTs can fill the ACT pipeline while DVE
            finishes this layer's cell update.
            """
            cfg = LAYERS[name]
            cls = "A" if name in ("enc0", "dec0") else "B"
            kin = cfg["kin"]
            gates = [None] * 4
            for g in range(4):
                if t == 0 and g == 1:
                    continue  # forget gate unused when c == 0
                gps = ps.tile([H, SBW], F32, tag="gps", bufs=2,
                              name=f"gps_{name}_{t}_{g}")
                for c in range(N_CHUNKS):
                    nc.tensor.matmul(
                        gps[:, bass.ts(c, CHUNK)],
                        w_in[name][:, bass.ts(g, H)],
                        rhs_in[:kin, bass.ts(c, CHUNK)],
                        start=True, stop=(t == 0))
                if t > 0:
                    for c in range(N_CHUNKS):
                        nc.tensor.matmul(
                            gps[:, bass.ts(c, CHUNK)],
                            w_rec[name][:, bass.ts(g, H)],
                            h_prev[:, bass.ts(c, CHUNK)],
                            start=False, stop=True)
                gate = gpool.tile([H, SBW], F16, tag=f"g{g}_{cls}",
                                  name=f"gate{g}_{name}_{t}")
                bias_arg = w_bias[name][:, g:g + 1] if cfg["act_bias"] else 0.0
                nc.scalar.activation(out=gate, in_=gps, func=GATE_FUNCS[g],
                                     bias=bias_arg)
                gates[g] = gate
            if t == 0:
                c_tile = cpool.tile([H, SBW], F16, tag=f"c_{name}",
                                    name=f"c_{name}_{t}")
                nc.vector.tensor_tensor(c_tile, gates[0], gates[2], MULT)
            else:
                u = tpool.tile([H, SBW], F16, tag=f"u_{cls}", bufs=2,
                               name=f"u_{name}_{t}")
                nc.vector.tensor_tensor(u, gates[0], gates[2], MULT)
                nc.vector.tensor_tensor(c_tile, gates[1], c_tile, MULT)
                nc.vector.tensor_tensor(c_tile, c_tile, u, ADD)
            return gates[3], c_tile

        def emit_outproj(sb, col0, td, htile):
            ops_t = ps.tile([H, SBW], F32, tag="gps", bufs=2,
                            name=f"op_{sb}_{td}")
            for c in range(N_CHUNKS):
                nc.tensor.matmul(ops_t[:D, bass.ts(c, CHUNK)], w_out,
                                 htile[:, bass.ts(c, CHUNK)],
                                 start=True, stop=True)
            o_t = opool.tile([D, SBW], F32, tag="o", name=f"o_{sb}_{td}")
            nc.vector.tensor_scalar_add(o_t, ops_t[:D, :], b_out)
            nc.sync.dma_start(out=out_ext[td, :, col0:col0 + SBW], in_=o_t)

        def step_tau_h(name, t, o_gate, c_tile, h_tag, h_bufs,
                       offload=False):
            cls = "A" if name in ("enc0", "dec0") else "B"
            tau = tpool.tile([H, SBW], F16, tag=f"tau_{cls}", bufs=2,
                             name=f"tau_{name}_{t}")
            nc.scalar.activation(out=tau, in_=c_tile, func=TANH)
            h_new = hpool.tile([H, SBW], F16, tag=h_tag, bufs=h_bufs,
                               name=f"h_{name}_{t}")
            nc.vector.tensor_tensor(h_new, o_gate, tau, MULT)
            return h_new

        # Software-pipelined over super-batches: while sb k runs its decoder,
        # sb k+1 runs its encoder, so the ACT engine always has a second
        # stream of gate work during latent/out-projection/warm-up phases.
        xq = {}
        x_order = [(s, t) for s in range(N_SB) for t in range(T)]
        x_state = [0]

        def x_ensure(upto_idx):
            while x_state[0] <= min(upto_idx, len(x_order) - 1):
                sbi, ti = x_order[x_state[0]]
                xt = xpool.tile([D + 1, SBW], F16, tag="x",
                                name=f"x_{sbi}_{ti}")
                nc.sync.dma_start(
                    out=xt, in_=x_ext[ti, :, sbi * SBW:(sbi + 1) * SBW])
                xq[(sbi, ti)] = xt
                x_state[0] += 1

        def emit_latent(sb, h1_tile):
            z_t = zpool.tile([L + 1, SBW], F16, tag="z", name=f"z_{sb}")
            lat_ps = ps.tile([H, SBW], F32, tag="gps", bufs=2,
                             name=f"lat_{sb}")
            for c in range(N_CHUNKS):
                nc.tensor.matmul(lat_ps[:L, bass.ts(c, CHUNK)], w_lat,
                                 h1_tile[:, bass.ts(c, CHUNK)],
                                 start=True, stop=True)
            nc.vector.tensor_scalar_add(z_t[:L, :], lat_ps[:L, :], b_lat)
            nc.vector.memset(z_t[L:L + 1, :], 1.0)
            return z_t

        pend_h1 = None
        tw = None
        for phase in range(N_SB + 1):
            enc_sb = phase if phase < N_SB else None
            dec_sb = phase - 1 if phase >= 1 else None
            z_dec = None

            if enc_sb is not None:
                ys = [None] * T
                h0 = c0 = h1 = c1 = None
            if dec_sb is not None:
                dcol0 = dec_sb * SBW
                d1 = [None] * T
                hd0 = cd0 = hd1 = cd1 = None
                op_queue = []

            for slot in range(T + 1):
                # Interleave: each layer's tanh/h tail is emitted between the
                # other layers' gate blocks so every h is ready well before
                # the next slot's recurrent matmuls need it, and the ACT
                # engine never drains its queue. The (one-slot-delayed)
                # out-projection sits mid-slot so its DVE-paced PSUM-ring
                # retirement never gates a slot boundary.
                o0 = o1 = od0 = od1 = None
                if enc_sb is not None:
                    if slot < T:
                        x_ensure(enc_sb * T + slot + 2)
                        x_t = xq.pop((enc_sb, slot))
                        o0, c0 = step_gates("enc0", slot, x_t, h0, c0)
                    if slot >= 1:
                        o1, c1 = step_gates("enc1", slot - 1, ys[slot - 1],
                                            h1, c1)
                if tw is None:
                    tw = load_tail_weights()
                    w_lat, b_lat = tw["w_lat"], tw["b_lat"]
                    w_out, b_out = tw["w_out"], tw["b_out"]
                if o0 is not None:
                    h0 = step_tau_h("enc0", slot, o0, c0, "y0", 3,
                                    offload=(enc_sb >= 1))
                    ys[slot] = h0
                if dec_sb is not None and slot == 0:
                    # latent for this phase's decoder, emitted after the new
                    # encoder's first gates so those cover its latency
                    z_dec = emit_latent(dec_sb, pend_h1)
                if dec_sb is not None and slot < T:
                    od0, cd0 = step_gates("dec0", slot, z_dec, hd0, cd0)
                if o1 is not None:
                    h1 = step_tau_h("enc1", slot - 1, o1, c1, "h1", 2)
                if dec_sb is not None and op_queue and (
                        len(op_queue) > 1 or slot == T):
                    emit_outproj(dec_sb, dcol0, *op_queue.pop(0))
                if dec_sb is not None and slot >= 1:
                    od1, cd1 = step_gates("dec1", slot - 1, d1[slot - 1],
                                          hd1, cd1)
                if od0 is not None:
                    hd0 = step_tau_h("dec0", slot, od0, cd0, "y1", 3,
                                     offload=True)
                    d1[slot] = hd0
                if od1 is not None:
                    hd1 = step_tau_h("dec1", slot - 1, od1, cd1, "d2", 3)
                    op_queue.append((slot - 1, hd1))

            if dec_sb is not None:
                for td, htile in op_queue:
                    emit_outproj(dec_sb, dcol0, td, htile)
            if enc_sb is not None:
                pend_h1 = h1

    nc.finalize()
    return nc


def _prep_inputs(inputs):
    """Host-side: transpose/pack fp32 inputs into per-core device arrays."""
    x = inputs["x"]
    xt = np.ascontiguousarray(np.transpose(x, (1, 2, 0)))   # [T, D, B]
    ones = np.ones((T, 1, B), np.float32)
    xt = np.concatenate([xt, ones], axis=1).astype(np.float16)

    common = {}
    for name in LAYERS:
        Wih = inputs[f"{name}_Wih"]
        Whh = inputs[f"{name}_Whh"]
        bsum = (inputs[f"{name}_bih"] + inputs[f"{name}_bhh"]).astype(np.float32)
        w_in = Wih.T.astype(np.float32)                      # [Din, 4H]
        if not LAYERS[name]["act_bias"]:
            w_in = np.concatenate([w_in, bsum[None, :]], axis=0)
        common[f"{name}_w_in"] = w_in.astype(np.float16)
        common[f"{name}_w_rec"] = Whh.T.astype(np.float16)
        if LAYERS[name]["act_bias"]:
            common[f"{name}_bias"] = np.ascontiguousarray(
                bsum.reshape(4, H).T)                        # [H, 4] fp32
    common["w_lat"] = inputs["W_lat"].T.astype(np.float16)   # [H, L]
    common["b_lat"] = inputs["b_lat"].reshape(L, 1).astype(np.float32)
    common["w_out"] = inputs["W_out"].T.astype(np.float16)   # [H, D]
    common["b_out"] = inputs["b_out"].reshape(D, 1).astype(np.float32)

    in_maps = []
    for core in range(N_CORES):
        m = dict(common)
        sl = slice(core * B_CORE, (core + 1) * B_CORE)
        m["x"] = np.ascontiguousarray(xt[:, :, sl])
        in_maps.append(m)
    return in_maps


def bench(inputs, reps: int = 8, reuse_nc=None):
    """Time repeated on-device executions (inputs device-resident, outputs
    left on device). Returns (best_seconds, all_times, outputs_of_first_run).
    """
    import jax
    from jax.sharding import Mesh, NamedSharding, PartitionSpec
    from jax.experimental.shard_map import shard_map
    from concourse import bass2jax
    from concourse.bass2jax import _bass_exec_p, partition_id_tensor

    bass2jax.install_neuronx_cc_hook()
    nc = reuse_nc if reuse_nc is not None else _build_kernel()
    in_maps = _prep_inputs(inputs)
    n_cores = N_CORES

    partition_name = nc.partition_id_tensor.name if nc.partition_id_tensor else None
    in_names, out_names, out_avals, zero_outs = [], [], [], []
    for alloc in nc.m.functions[0].allocations:
        if not isinstance(alloc, mybir.MemoryLocationSet):
            continue
        name = alloc.memorylocations[0].name
        if alloc.kind == "ExternalInput":
            if name != partition_name:
                in_names.append(name)
        elif alloc.kind == "ExternalOutput":
            out_names.append(name)
            out_avals.append(
                jax.core.ShapedArray(tuple(alloc.tensor_shape),
                                     mybir.dt.np(alloc.dtype)))
            zero_outs.append(
                np.zeros(tuple(alloc.tensor_shape), mybir.dt.np(alloc.dtype)))
    n_params = len(in_names)
    n_outs = len(out_names)
    all_in_names = in_names + out_names + ([partition_name] if partition_name else [])
    donate = tuple(range(n_params, n_params + n_outs))

    def _body(*args):
        operands = list(args)
        if partition_name is not None:
            operands.append(partition_id_tensor())
        return tuple(_bass_exec_p.bind(
            *operands, out_avals=tuple(out_avals), in_names=tuple(all_in_names),
            out_names=tuple(out_names), lowering_input_output_aliases=(),
            sim_require_finite=True, sim_require_nnan=True, nc=nc))

    devices = jax.devices()[:n_cores]
    mesh = Mesh(np.asarray(devices), ("core",))
    in_specs = (PartitionSpec("core"),) * (n_params + n_outs)
    out_specs = (PartitionSpec("core"),) * n_outs
    sharded = jax.jit(
        shard_map(_body, mesh=mesh, in_specs=in_specs, out_specs=out_specs,
                  check_rep=False),
        donate_argnums=donate, keep_unused=True)

    shard = NamedSharding(mesh, PartitionSpec("core"))
    concat_in = [
        jax.device_put(
            np.concatenate([np.asarray(in_maps[c][nm]) for c in range(n_cores)], 0),
            shard)
        for nm in in_names
    ]
    def fresh_zeros():
        return [jax.device_put(
                    np.zeros((n_cores * z.shape[0], *z.shape[1:]), z.dtype), shard)
                for z in zero_outs]

    # warm-up (compile)
    outs0 = sharded(*concat_in, *fresh_zeros())
    jax.block_until_ready(outs0)

    zero_sets = [fresh_zeros() for _ in range(reps)]
    jax.block_until_ready(zero_sets)
    times = []
    for r in range(reps):
        t0 = time.perf_counter()
        outs = sharded(*concat_in, *zero_sets[r])
        jax.block_until_ready(outs)
        times.append(time.perf_counter() - t0)
    return min(times), times, outs0


def kernel(**inputs) -> np.ndarray:
    global _last_results
    trace = bool(int(os.environ.get("BASS_LSTM_TRACE", "0")))
    nc = _build_kernel(trace)
    in_maps = _prep_inputs(inputs)
    res = run_bass_kernel_spmd(nc, in_maps, core_ids=list(range(N_CORES)),
                               trace=trace)
    _last_results = res
    outs = [res.results[c]["out"] for c in range(N_CORES)]   # [T, D, B_CORE]
    full = np.concatenate(outs, axis=2)                      # [T, D, B]
    return np.ascontiguousarray(np.transpose(full, (2, 0, 1)))  # [B, T, D]


# revision 37
# speedup vs baseline: 1.1286x; 1.0364x over previous
"""Trainium2 Bass kernel for a 4-layer LSTM autoencoder.

Contract: kernel(**inputs) takes the FULL fp32 inputs (B=65536) and returns
the full [B, T, D] fp32 reconstruction. Internally: pure data parallelism —
the batch is sharded across 8 NeuronCores; weights are replicated.

Device-side layout: everything is stored transposed, [feature=partitions,
batch=free]. Gates are computed as W_g @ x (+ W_hg @ h) with the batch
streaming through the PE array, so the recurrent state h never needs an
on-chip transpose. The host pre-transposes x and post-transposes the output.

The kernel is activation-engine bound (4 gate LUTs + tanh(c) per cell), so
the design keeps the ACT engine saturated with maximal-width instructions:

  - per layer-step (2048 columns): each gate gets a dedicated [128, 2048]
    fp32 PSUM tile (4 banks; 2 tiles ping-pong = all 8 banks). 4 input
    matmuls (start=True) then 4 recurrent matmuls fill it; ONE 2048-wide
    ACT (sigmoid/tanh) drains it to an fp16 SBUF gate tile. Biases are
    folded into the input matmul via an appended ones-row (enc0/dec0) or
    applied via the ACT per-partition bias operand (enc1/dec1).
  - DVE (all fp16, 2x perf mode): u = i*g, c = f*c + u, h = o*tanh(c).
  - latent/output-projection bias adds run on DVE (tensor_scalar_add with a
    per-partition bias AP) instead of ACT; the output projection is batched
    into an end phase over stored dec1 hiddens so it never steals PSUM or
    ACT slots from the recurrence.
Encoder layers (and decoder layers) are pipelined with a 1-step skew so the
recurrence latency of one layer hides under the other layer's ACT work.
"""

import os
import sys
import time
from contextlib import ExitStack

import numpy as np

sys.path.insert(0, "/opt/trn_rl_repo")

import concourse.bass as bass  # noqa: E402
import concourse.tile as tile  # noqa: E402
from concourse import bacc, mybir  # noqa: E402
from concourse.bass_utils import run_bass_kernel_spmd  # noqa: E402

F32 = mybir.dt.float32
F16 = mybir.dt.float16
SIG = mybir.ActivationFunctionType.Sigmoid
TANH = mybir.ActivationFunctionType.Tanh
MULT = mybir.AluOpType.mult
ADD = mybir.AluOpType.add
MIN = mybir.AluOpType.min
MAX = mybir.AluOpType.max
DIV = mybir.AluOpType.divide

B, T, D, H, L = 65536, 8, 60, 128, 64
N_CORES = 8
B_CORE = B // N_CORES        # 8192
SBW = 2048                   # super-batch width (columns in flight)
N_SB = B_CORE // SBW         # 4
CHUNK = 512                  # matmul moving-operand width (one PSUM bank)
N_CHUNKS = SBW // CHUNK      # 4

# layer descriptors: (input feature dim incl. ones-row, bias-in-ACT?)
LAYERS = {
    "enc0": dict(kin=D + 1, act_bias=False),
    "enc1": dict(kin=H, act_bias=True),
    "dec0": dict(kin=L + 1, act_bias=False),
    "dec1": dict(kin=H, act_bias=True),
}
GATE_FUNCS = [SIG, SIG, TANH, SIG]  # PyTorch gate order: i, f, g, o

_last_results = None  # set by kernel(); test harness reads exec_time_ns


def _build_kernel(trace: bool = False):
    nc = bacc.Bacc("TRN2", target_bir_lowering=False, debug=False,
                   num_devices=N_CORES)

    x_ext = nc.dram_tensor("x", [T, D + 1, B_CORE], F16, kind="ExternalInput").ap()
    out_ext = nc.dram_tensor("out", [T, D, B_CORE], F32, kind="ExternalOutput").ap()

    w_in_ext, w_rec_ext, bias_ext = {}, {}, {}
    for name, cfg in LAYERS.items():
        w_in_ext[name] = nc.dram_tensor(
            f"{name}_w_in", [cfg["kin"], 4 * H], F16, kind="ExternalInput").ap()
        w_rec_ext[name] = nc.dram_tensor(
            f"{name}_w_rec", [H, 4 * H], F16, kind="ExternalInput").ap()
        if cfg["act_bias"]:
            bias_ext[name] = nc.dram_tensor(
                f"{name}_bias", [H, 4], F32, kind="ExternalInput").ap()
    w_lat_ext = nc.dram_tensor("w_lat", [H, L], F16, kind="ExternalInput").ap()
    b_lat_ext = nc.dram_tensor("b_lat", [L, 1], F32, kind="ExternalInput").ap()
    w_out_ext = nc.dram_tensor("w_out", [H, D], F16, kind="ExternalInput").ap()
    b_out_ext = nc.dram_tensor("b_out", [D, 1], F32, kind="ExternalInput").ap()

    with tile.TileContext(nc) as tc, ExitStack() as ctx:
        weights = ctx.enter_context(tc.tile_pool(name="weights", bufs=1))
        xpool = ctx.enter_context(tc.tile_pool(name="xpool", bufs=3))
        ypool = ctx.enter_context(tc.tile_pool(name="ypool", bufs=1))
        hpool = ctx.enter_context(tc.tile_pool(name="hpool", bufs=1))
        cpool = ctx.enter_context(tc.tile_pool(name="cpool", bufs=1))
        gpool = ctx.enter_context(tc.tile_pool(name="gpool", bufs=1))
        tpool = ctx.enter_context(tc.tile_pool(name="tpool", bufs=1))
        zpool = ctx.enter_context(tc.tile_pool(name="zpool", bufs=2))
        opool = ctx.enter_context(tc.tile_pool(name="opool", bufs=2))
        ps = ctx.enter_context(tc.tile_pool(name="ps", bufs=1, space="PSUM"))

        # ---- load weights once (encoder weights first; the rest is emitted
        # after the first slot so the HWDGE queue doesn't delay x[0]) ----
        w_in, w_rec, w_bias = {}, {}, {}

        def load_layer_weights(name):
            cfg = LAYERS[name]
            w_in[name] = weights.tile([cfg["kin"], 4 * H], F16,
                                      tag=f"wi_{name}", name=f"wi_{name}")
            nc.sync.dma_start(out=w_in[name], in_=w_in_ext[name][:, :])
            w_rec[name] = weights.tile([H, 4 * H], F16,
                                       tag=f"wr_{name}", name=f"wr_{name}")
            nc.sync.dma_start(out=w_rec[name], in_=w_rec_ext[name][:, :])
            if cfg["act_bias"]:
                w_bias[name] = weights.tile([H, 4], F32,
                                            tag=f"wb_{name}", name=f"wb_{name}")
                nc.sync.dma_start(out=w_bias[name], in_=bias_ext[name][:, :])

        load_layer_weights("enc0")

        def load_tail_weights():
            load_layer_weights("enc1")
            load_layer_weights("dec0")
            load_layer_weights("dec1")
            tw = {}
            tw["w_lat"] = weights.tile([H, L], F16, tag="w_lat", name="w_lat")
            nc.sync.dma_start(out=tw["w_lat"], in_=w_lat_ext[:, :])
            tw["b_lat"] = weights.tile([L, 1], F32, tag="b_lat", name="b_lat")
            nc.sync.dma_start(out=tw["b_lat"], in_=b_lat_ext[:, :])
            tw["w_out"] = weights.tile([H, D], F16, tag="w_out", name="w_out")
            nc.sync.dma_start(out=tw["w_out"], in_=w_out_ext[:, :])
            tw["b_out"] = weights.tile([D, 1], F32, tag="b_out", name="b_out")
            nc.sync.dma_start(out=tw["b_out"], in_=b_out_ext[:, :])
            return tw

        def step_gates(name, t, rhs_in, h_prev, c_tile):
            """Gate matmuls + gate ACTs + DVE cell update for one layer-step.

            Per gate: one [128, SBW] PSUM tile (input matmuls first — they
            only need rhs_in — then recurrent), drained by a single SBW-wide
            ACT into an fp16 gate tile. Returns (o_gate, c_tile). The tanh(c)
            + h = o*tanh(c) tail is emitted separately (step_tau_h) so the
            other layer's gate ACTs can fill the ACT pipeline while DVE
            finishes this layer's cell update.
            """
            cfg = LAYERS[name]
            cls = "A" if name in ("enc0", "dec0") else "B"
            kin = cfg["kin"]
            gates = [None] * 4
            for g in range(4):
                if t == 0 and g == 1:
                    continue  # forget gate unused when c == 0
                gps = ps.tile([H, SBW], F32, tag="gps", bufs=2,
                              name=f"gps_{name}_{t}_{g}")
                for c in range(N_CHUNKS):
                    nc.tensor.matmul(
                        gps[:, bass.ts(c, CHUNK)],
                        w_in[name][:, bass.ts(g, H)],
                        rhs_in[:kin, bass.ts(c, CHUNK)],
                        start=True, stop=(t == 0))
                if t > 0:
                    for c in range(N_CHUNKS):
                        nc.tensor.matmul(
                            gps[:, bass.ts(c, CHUNK)],
                            w_rec[name][:, bass.ts(g, H)],
                            h_prev[:, bass.ts(c, CHUNK)],
                            start=False, stop=True)
                gate = gpool.tile([H, SBW], F16, tag=f"g{g}_{cls}",
                                  name=f"gate{g}_{name}_{t}")
                bias_arg = w_bias[name][:, g:g + 1] if cfg["act_bias"] else 0.0
                nc.scalar.activation(out=gate, in_=gps, func=GATE_FUNCS[g],
                                     bias=bias_arg)
                gates[g] = gate
            if t == 0:
                c_tile = cpool.tile([H, SBW], F16, tag=f"c_{name}",
                                    name=f"c_{name}_{t}")
                nc.vector.tensor_tensor(c_tile, gates[0], gates[2], MULT)
            else:
                u = tpool.tile([H, SBW], F16, tag=f"u_{cls}", bufs=2,
                               name=f"u_{name}_{t}")
                nc.vector.tensor_tensor(u, gates[0], gates[2], MULT)
                nc.vector.tensor_tensor(c_tile, gates[1], c_tile, MULT)
                nc.vector.tensor_tensor(c_tile, c_tile, u, ADD)
            return gates[3], c_tile

        def emit_outproj(sb, col0, td, htile):
            ops_t = ps.tile([H, SBW], F32, tag="gps", bufs=2,
                            name=f"op_{sb}_{td}")
            for c in range(N_CHUNKS):
                nc.tensor.matmul(ops_t[:D, bass.ts(c, CHUNK)], w_out,
                                 htile[:, bass.ts(c, CHUNK)],
                                 start=True, stop=True)
            o_t = opool.tile([D, SBW], F32, tag="o", name=f"o_{sb}_{td}")
            nc.vector.tensor_scalar_add(o_t, ops_t[:D, :], b_out)
            nc.sync.dma_start(out=out_ext[td, :, col0:col0 + SBW], in_=o_t)

        def step_tau_h(name, t, o_gate, c_tile, h_tag, h_bufs):
            cls = "A" if name in ("enc0", "dec0") else "B"
            tau = tpool.tile([H, SBW], F16, tag=f"tau_{cls}", bufs=2,
                             name=f"tau_{name}_{t}")
            nc.scalar.activation(out=tau, in_=c_tile, func=TANH)
            h_new = hpool.tile([H, SBW], F16, tag=h_tag, bufs=h_bufs,
                               name=f"h_{name}_{t}")
            nc.vector.tensor_tensor(h_new, o_gate, tau, MULT)
            return h_new

        # tanh(c) as a clamped Pade[5/4] rational on DVE (max err ~2.4e-3 in
        # fp16) for a subset of steps, to offload the saturated ACT engine.
        # Emitted in two pieces at different slot positions so the chain
        # never head-of-line-blocks another stream's DVE cell update.
        def pade_begin(name, t, c_tile):
            cl = tpool.tile([H, SBW], F16, tag=f"cl_{name}", bufs=1,
                            name=f"cl_{name}_{t}")
            nc.vector.tensor_scalar(out=cl, in0=c_tile, scalar1=3.5,
                                    scalar2=-3.5, op0=MIN, op1=MAX)
            u2 = tpool.tile([H, SBW], F16, tag=f"u2_{name}", bufs=1,
                            name=f"u2_{name}_{t}")
            nc.vector.tensor_tensor(u2, cl, cl, MULT)
            qq = tpool.tile([H, SBW], F16, tag=f"qq_{name}", bufs=1,
                            name=f"qq_{name}_{t}")
            nc.vector.tensor_scalar(out=qq, in0=u2, scalar1=1.0 / 63.0,
                                    scalar2=4.0 / 9.0, op0=MULT, op1=ADD)
            nc.vector.tensor_tensor(qq, qq, u2, MULT)
            nc.vector.tensor_scalar(out=qq, in0=qq, scalar1=1.0,
                                    scalar2=None, op0=ADD)
            with nc.allow_low_precision("fp16 pade tanh reciprocal"):
                nc.vector.reciprocal(qq, qq)
            return (name, t, cl, u2, qq)

        def pade_end(state, o_gate, h_tag, h_bufs):
            name, t, cl, u2, qq = state
            pp = tpool.tile([H, SBW], F16, tag=f"pp_{name}", bufs=1,
                            name=f"pp_{name}_{t}")
            nc.vector.tensor_scalar(out=pp, in0=u2, scalar1=1.0 / 945.0,
                                    scalar2=1.0 / 9.0, op0=MULT, op1=ADD)
            nc.vector.tensor_tensor(pp, pp, u2, MULT)
            nc.vector.tensor_scalar(out=pp, in0=pp, scalar1=1.0,
                                    scalar2=None, op0=ADD)
            nc.vector.tensor_tensor(pp, pp, cl, MULT)
            nc.vector.tensor_tensor(cl, pp, qq, MULT)  # cl is dead; reuse
            h_new = hpool.tile([H, SBW], F16, tag=h_tag, bufs=h_bufs,
                               name=f"h_{name}_{t}")
            nc.vector.tensor_tensor(h_new, o_gate, cl, MULT)
            return h_new

        # Software-pipelined over super-batches: while sb k runs its decoder,
        # sb k+1 runs its encoder, so the ACT engine always has a second
        # stream of gate work during latent/out-projection/warm-up phases.
        xq = {}
        x_order = [(s, t) for s in range(N_SB) for t in range(T)]
        x_state = [0]

        def x_ensure(upto_idx):
            while x_state[0] <= min(upto_idx, len(x_order) - 1):
                sbi, ti = x_order[x_state[0]]
                xt = xpool.tile([D + 1, SBW], F16, tag="x",
                                name=f"x_{sbi}_{ti}")
                nc.sync.dma_start(
                    out=xt, in_=x_ext[ti, :, sbi * SBW:(sbi + 1) * SBW])
                xq[(sbi, ti)] = xt
                x_state[0] += 1

        def emit_latent(sb, h1_tile):
            z_t = zpool.tile([L + 1, SBW], F16, tag="z", name=f"z_{sb}")
            lat_ps = ps.tile([H, SBW], F32, tag="gps", bufs=2,
                             name=f"lat_{sb}")
            for c in range(N_CHUNKS):
                nc.tensor.matmul(lat_ps[:L, bass.ts(c, CHUNK)], w_lat,
                                 h1_tile[:, bass.ts(c, CHUNK)],
                                 start=True, stop=True)
            nc.vector.tensor_scalar_add(z_t[:L, :], lat_ps[:L, :], b_lat)
            nc.vector.memset(z_t[L:L + 1, :], 1.0)
            return z_t

        pend_h1 = None
        tw = None
        op_carry = []
        for phase in range(N_SB + 1):
            enc_sb = phase if phase < N_SB else None
            dec_sb = phase - 1 if phase >= 1 else None
            z_dec = None

            if enc_sb is not None:
                ys = [None] * T
                h0 = c0 = h1 = c1 = None
            if dec_sb is not None:
                dcol0 = dec_sb * SBW
                d1 = [None] * T
                hd0 = cd0 = hd1 = cd1 = None
                op_queue = []

            for slot in range(T + 1):
                # Interleave: each layer's tanh/h tail is emitted between the
                # other layers' gate blocks so every h is ready well before
                # the next slot's recurrent matmuls need it, and the ACT
                # engine never drains its queue. The (one-slot-delayed)
                # out-projection sits mid-slot so its DVE-paced PSUM-ring
                # retirement never gates a slot boundary.
                o0 = o1 = od0 = od1 = None
                if enc_sb is not None:
                    if slot < T:
                        x_ensure(enc_sb * T + slot + 2)
                        x_t = xq.pop((enc_sb, slot))
                        o0, c0 = step_gates("enc0", slot, x_t, h0, c0)
                    if slot >= 1:
                        o1, c1 = step_gates("enc1", slot - 1, ys[slot - 1],
                                            h1, c1)
                if tw is None:
                    tw = load_tail_weights()
                    w_lat, b_lat = tw["w_lat"], tw["b_lat"]
                    w_out, b_out = tw["w_out"], tw["b_out"]
                off_e0 = False  # tau offload net-negative under the current
                off_d0 = False  # schedule; pade path kept dormant
                pe0 = None
                if o0 is not None:
                    if off_e0:
                        pe0 = pade_begin("enc0", slot, c0)
                    else:
                        h0 = step_tau_h("enc0", slot, o0, c0, "y0", 3)
                        ys[slot] = h0
                if dec_sb is not None and slot == 0:
                    # latent for this phase's decoder, emitted after the new
                    # encoder's first gates so those cover its latency
                    z_dec = emit_latent(dec_sb, pend_h1)
                pd0 = None
                if dec_sb is not None and slot < T:
                    od0, cd0 = step_gates("dec0", slot, z_dec, hd0, cd0)
                    if off_d0:
                        pd0 = pade_begin("dec0", slot, cd0)
                    else:
                        hd0 = step_tau_h("dec0", slot, od0, cd0, "y1", 3)
                        d1[slot] = hd0
                if o1 is not None:
                    h1 = step_tau_h("enc1", slot - 1, o1, c1, "h1", 2)
                if op_carry:
                    emit_outproj(*op_carry.pop(0))
                if dec_sb is not None and op_queue and (
                        len(op_queue) > 1 or slot == T):
                    emit_outproj(dec_sb, dcol0, *op_queue.pop(0))
                if dec_sb is not None and slot >= 1:
                    od1, cd1 = step_gates("dec1", slot - 1, d1[slot - 1],
                                          hd1, cd1)
                if pe0 is not None:
                    h0 = pade_end(pe0, o0, "y0", 3)
                    ys[slot] = h0
                if pd0 is not None:
                    hd0 = pade_end(pd0, od0, "y1", 3)
                    d1[slot] = hd0
                if od1 is not None:
                    hd1 = step_tau_h("dec1", slot - 1, od1, cd1, "d2", 3)
                    op_queue.append((slot - 1, hd1))

            if dec_sb is not None:
                # leftover out-projections flush in the next phase's early
                # slots so their PSUM-ring retirement never gates the
                # phase boundary
                op_carry = [(dec_sb, dcol0, td, htile)
                            for td, htile in op_queue]
            if enc_sb is not None:
                pend_h1 = h1
        for args in op_carry:
            emit_outproj(*args)

    nc.finalize()
    return nc


def _prep_inputs(inputs):
    """Host-side: transpose/pack fp32 inputs into per-core device arrays."""
    x = inputs["x"]
    xt = np.ascontiguousarray(np.transpose(x, (1, 2, 0)))   # [T, D, B]
    ones = np.ones((T, 1, B), np.float32)
    xt = np.concatenate([xt, ones], axis=1).astype(np.float16)

    common = {}
    for name in LAYERS:
        Wih = inputs[f"{name}_Wih"]
        Whh = inputs[f"{name}_Whh"]
        bsum = (inputs[f"{name}_bih"] + inputs[f"{name}_bhh"]).astype(np.float32)
        w_in = Wih.T.astype(np.float32)                      # [Din, 4H]
        if not LAYERS[name]["act_bias"]:
            w_in = np.concatenate([w_in, bsum[None, :]], axis=0)
        common[f"{name}_w_in"] = w_in.astype(np.float16)
        common[f"{name}_w_rec"] = Whh.T.astype(np.float16)
        if LAYERS[name]["act_bias"]:
            common[f"{name}_bias"] = np.ascontiguousarray(
                bsum.reshape(4, H).T)                        # [H, 4] fp32
    common["w_lat"] = inputs["W_lat"].T.astype(np.float16)   # [H, L]
    common["b_lat"] = inputs["b_lat"].reshape(L, 1).astype(np.float32)
    common["w_out"] = inputs["W_out"].T.astype(np.float16)   # [H, D]
    common["b_out"] = inputs["b_out"].reshape(D, 1).astype(np.float32)

    in_maps = []
    for core in range(N_CORES):
        m = dict(common)
        sl = slice(core * B_CORE, (core + 1) * B_CORE)
        m["x"] = np.ascontiguousarray(xt[:, :, sl])
        in_maps.append(m)
    return in_maps


def bench(inputs, reps: int = 8, reuse_nc=None):
    """Time repeated on-device executions (inputs device-resident, outputs
    left on device). Returns (best_seconds, all_times, outputs_of_first_run).
    """
    import jax
    from jax.sharding import Mesh, NamedSharding, PartitionSpec
    from jax.experimental.shard_map import shard_map
    from concourse import bass2jax
    from concourse.bass2jax import _bass_exec_p, partition_id_tensor

    bass2jax.install_neuronx_cc_hook()
    nc = reuse_nc if reuse_nc is not None else _build_kernel()
    in_maps = _prep_inputs(inputs)
    n_cores = N_CORES

    partition_name = nc.partition_id_tensor.name if nc.partition_id_tensor else None
    in_names, out_names, out_avals, zero_outs = [], [], [], []
    for alloc in nc.m.functions[0].allocations:
        if not isinstance(alloc, mybir.MemoryLocationSet):
            continue
        name = alloc.memorylocations[0].name
        if alloc.kind == "ExternalInput":
            if name != partition_name:
                in_names.append(name)
        elif alloc.kind == "ExternalOutput":
            out_names.append(name)
            out_avals.append(
                jax.core.ShapedArray(tuple(alloc.tensor_shape),
                                     mybir.dt.np(alloc.dtype)))
            zero_outs.append(
                np.zeros(tuple(alloc.tensor_shape), mybir.dt.np(alloc.dtype)))
    n_params = len(in_names)
    n_outs = len(out_names)
    all_in_names = in_names + out_names + ([partition_name] if partition_name else [])
    donate = tuple(range(n_params, n_params + n_outs))

    def _body(*args):
        operands = list(args)
        if partition_name is not None:
            operands.append(partition_id_tensor())
        return tuple(_bass_exec_p.bind(
            *operands, out_avals=tuple(out_avals), in_names=tuple(all_in_names),
            out_names=tuple(out_names), lowering_input_output_aliases=(),
            sim_require_finite=True, sim_require_nnan=True, nc=nc))

    devices = jax.devices()[:n_cores]
    mesh = Mesh(np.asarray(devices), ("core",))
    in_specs = (PartitionSpec("core"),) * (n_params + n_outs)
    out_specs = (PartitionSpec("core"),) * n_outs
    sharded = jax.jit(
        shard_map(_body, mesh=mesh, in_specs=in_specs, out_specs=out_specs,
                  check_rep=False),
        donate_argnums=donate, keep_unused=True)

    shard = NamedSharding(mesh, PartitionSpec("core"))
    concat_in = [
        jax.device_put(
            np.concatenate([np.asarray(in_maps[c][nm]) for c in range(n_cores)], 0),
            shard)
        for nm in in_names
    ]
    def fresh_zeros():
        return [jax.device_put(
                    np.zeros((n_cores * z.shape[0], *z.shape[1:]), z.dtype), shard)
                for z in zero_outs]

    # warm-up (compile)
    outs0 = sharded(*concat_in, *fresh_zeros())
    jax.block_until_ready(outs0)

    zero_sets = [fresh_zeros() for _ in range(reps)]
    jax.block_until_ready(zero_sets)
    times = []
    for r in range(reps):
        t0 = time.perf_counter()
        outs = sharded(*concat_in, *zero_sets[r])
        jax.block_until_ready(outs)
        times.append(time.perf_counter() - t0)
    return min(times), times, outs0


def kernel(**inputs) -> np.ndarray:
    global _last_results
    trace = bool(int(os.environ.get("BASS_LSTM_TRACE", "0")))
    nc = _build_kernel(trace)
    in_maps = _prep_inputs(inputs)
    res = run_bass_kernel_spmd(nc, in_maps, core_ids=list(range(N_CORES)),
                               trace=trace)
    _last_results = res
    outs = [res.results[c]["out"] for c in range(N_CORES)]   # [T, D, B_CORE]
    full = np.concatenate(outs, axis=2)                      # [T, D, B]
    return np.ascontiguousarray(np.transpose(full, (2, 0, 1)))  # [B, T, D]


# revision 44
# speedup vs baseline: 1.1311x; 1.0022x over previous
"""Trainium2 Bass kernel for a 4-layer LSTM autoencoder.

Contract: kernel(**inputs) takes the FULL fp32 inputs (B=65536) and returns
the full [B, T, D] fp32 reconstruction. Internally: pure data parallelism —
the batch is sharded across 8 NeuronCores; weights are replicated.

Device-side layout: everything is stored transposed, [feature=partitions,
batch=free]. Gates are computed as W_g @ x (+ W_hg @ h) with the batch
streaming through the PE array, so the recurrent state h never needs an
on-chip transpose. The host pre-transposes x and post-transposes the output.

The kernel is activation-engine bound (4 gate LUTs + tanh(c) per cell), so
the design keeps the ACT engine saturated with maximal-width instructions:

  - per layer-step (2048 columns): each gate gets a dedicated [128, 2048]
    fp32 PSUM tile (4 banks; 2 tiles ping-pong = all 8 banks). 4 input
    matmuls (start=True) then 4 recurrent matmuls fill it; ONE 2048-wide
    ACT (sigmoid/tanh) drains it to an fp16 SBUF gate tile. Biases are
    folded into the input matmul via an appended ones-row (enc0/dec0) or
    applied via the ACT per-partition bias operand (enc1/dec1).
  - DVE (all fp16, 2x perf mode): u = i*g, c = f*c + u, h = o*tanh(c).
  - latent/output-projection bias adds run on DVE (tensor_scalar_add with a
    per-partition bias AP) instead of ACT; the output projection is batched
    into an end phase over stored dec1 hiddens so it never steals PSUM or
    ACT slots from the recurrence.
Encoder layers (and decoder layers) are pipelined with a 1-step skew so the
recurrence latency of one layer hides under the other layer's ACT work.
"""

import os
import sys
import time
from contextlib import ExitStack

import numpy as np

sys.path.insert(0, "/opt/trn_rl_repo")

import concourse.bass as bass  # noqa: E402
import concourse.tile as tile  # noqa: E402
from concourse import bacc, mybir  # noqa: E402
from concourse.bass_utils import run_bass_kernel_spmd  # noqa: E402

F32 = mybir.dt.float32
F16 = mybir.dt.float16
SIG = mybir.ActivationFunctionType.Sigmoid
TANH = mybir.ActivationFunctionType.Tanh
MULT = mybir.AluOpType.mult
ADD = mybir.AluOpType.add
MIN = mybir.AluOpType.min
MAX = mybir.AluOpType.max
DIV = mybir.AluOpType.divide

B, T, D, H, L = 65536, 8, 60, 128, 64
N_CORES = 8
B_CORE = B // N_CORES        # 8192
SBW = 2048                   # super-batch width (columns in flight)
N_SB = B_CORE // SBW         # 4
CHUNK = 512                  # matmul moving-operand width (one PSUM bank)
N_CHUNKS = SBW // CHUNK      # 4

# layer descriptors: (input feature dim incl. ones-row, bias-in-ACT?)
LAYERS = {
    "enc0": dict(kin=D + 1, act_bias=False),
    "enc1": dict(kin=H, act_bias=True),
    "dec0": dict(kin=L + 1, act_bias=False),
    "dec1": dict(kin=H, act_bias=True),
}
GATE_FUNCS = [SIG, SIG, TANH, SIG]  # PyTorch gate order: i, f, g, o

_last_results = None  # set by kernel(); test harness reads exec_time_ns


def _build_kernel(trace: bool = False):
    nc = bacc.Bacc("TRN2", target_bir_lowering=False, debug=False,
                   num_devices=N_CORES)

    x_ext = nc.dram_tensor("x", [T, D + 1, B_CORE], F16, kind="ExternalInput").ap()
    out_ext = nc.dram_tensor("out", [T, D, B_CORE], F32, kind="ExternalOutput").ap()

    w_in_ext, w_rec_ext, bias_ext = {}, {}, {}
    for name, cfg in LAYERS.items():
        w_in_ext[name] = nc.dram_tensor(
            f"{name}_w_in", [cfg["kin"], 4 * H], F16, kind="ExternalInput").ap()
        w_rec_ext[name] = nc.dram_tensor(
            f"{name}_w_rec", [H, 4 * H], F16, kind="ExternalInput").ap()
        if cfg["act_bias"]:
            bias_ext[name] = nc.dram_tensor(
                f"{name}_bias", [H, 4], F32, kind="ExternalInput").ap()
    w_lat_ext = nc.dram_tensor("w_lat", [H, L], F16, kind="ExternalInput").ap()
    b_lat_ext = nc.dram_tensor("b_lat", [L, 1], F32, kind="ExternalInput").ap()
    w_out_ext = nc.dram_tensor("w_out", [H, D], F16, kind="ExternalInput").ap()
    b_out_ext = nc.dram_tensor("b_out", [D, 1], F32, kind="ExternalInput").ap()

    with tile.TileContext(nc) as tc, ExitStack() as ctx:
        weights = ctx.enter_context(tc.tile_pool(name="weights", bufs=1))
        xpool = ctx.enter_context(tc.tile_pool(name="xpool", bufs=3))
        ypool = ctx.enter_context(tc.tile_pool(name="ypool", bufs=1))
        hpool = ctx.enter_context(tc.tile_pool(name="hpool", bufs=1))
        cpool = ctx.enter_context(tc.tile_pool(name="cpool", bufs=1))
        gpool = ctx.enter_context(tc.tile_pool(name="gpool", bufs=1))
        tpool = ctx.enter_context(tc.tile_pool(name="tpool", bufs=1))
        zpool = ctx.enter_context(tc.tile_pool(name="zpool", bufs=2))
        opool = ctx.enter_context(tc.tile_pool(name="opool", bufs=2))
        ps = ctx.enter_context(tc.tile_pool(name="ps", bufs=1, space="PSUM"))

        # ---- load weights once (encoder weights first; the rest is emitted
        # after the first slot so the HWDGE queue doesn't delay x[0]) ----
        w_in, w_rec, w_bias = {}, {}, {}

        def load_layer_weights(name):
            cfg = LAYERS[name]
            w_in[name] = weights.tile([cfg["kin"], 4 * H], F16,
                                      tag=f"wi_{name}", name=f"wi_{name}")
            nc.sync.dma_start(out=w_in[name], in_=w_in_ext[name][:, :])
            w_rec[name] = weights.tile([H, 4 * H], F16,
                                       tag=f"wr_{name}", name=f"wr_{name}")
            nc.sync.dma_start(out=w_rec[name], in_=w_rec_ext[name][:, :])
            if cfg["act_bias"]:
                w_bias[name] = weights.tile([H, 4], F32,
                                            tag=f"wb_{name}", name=f"wb_{name}")
                nc.sync.dma_start(out=w_bias[name], in_=bias_ext[name][:, :])

        load_layer_weights("enc0")

        def load_tail_weights():
            load_layer_weights("enc1")
            load_layer_weights("dec0")
            load_layer_weights("dec1")
            tw = {}
            tw["w_lat"] = weights.tile([H, L], F16, tag="w_lat", name="w_lat")
            nc.sync.dma_start(out=tw["w_lat"], in_=w_lat_ext[:, :])
            tw["b_lat"] = weights.tile([L, 1], F32, tag="b_lat", name="b_lat")
            nc.sync.dma_start(out=tw["b_lat"], in_=b_lat_ext[:, :])
            tw["w_out"] = weights.tile([H, D], F16, tag="w_out", name="w_out")
            nc.sync.dma_start(out=tw["w_out"], in_=w_out_ext[:, :])
            tw["b_out"] = weights.tile([D, 1], F32, tag="b_out", name="b_out")
            nc.sync.dma_start(out=tw["b_out"], in_=b_out_ext[:, :])
            return tw

        def step_gates(name, t, rhs_in, h_prev, c_tile, split_first=False):
            """Gate matmuls + gate ACTs + DVE cell update for one layer-step.

            Per gate: one [128, SBW] PSUM tile (input matmuls first — they
            only need rhs_in — then recurrent), drained by a single SBW-wide
            ACT into an fp16 gate tile. Returns (o_gate, c_tile). The tanh(c)
            + h = o*tanh(c) tail is emitted separately (step_tau_h) so the
            other layer's gate ACTs can fill the ACT pipeline while DVE
            finishes this layer's cell update.
            """
            cfg = LAYERS[name]
            cls = "A" if name in ("enc0", "dec0") else "B"
            kin = cfg["kin"]
            gates = [None] * 4
            for g in range(4):
                if t == 0 and g == 1:
                    continue  # forget gate unused when c == 0
                gate = gpool.tile([H, SBW], F16, tag=f"g{g}_{cls}",
                                  name=f"gate{g}_{name}_{t}")
                bias_arg = w_bias[name][:, g:g + 1] if cfg["act_bias"] else 0.0
                # The slot's first gate is split into two half-width PSUM
                # tiles: the first half's fill can begin one ring-retirement
                # earlier, hiding the fill latency at the slot boundary.
                n_parts = 2 if (split_first and g == 0) else 1
                pw = SBW // n_parts
                for part in range(n_parts):
                    gps = ps.tile([H, pw], F32, tag="gps", bufs=2,
                                  name=f"gps_{name}_{t}_{g}_{part}")
                    for c in range(pw // CHUNK):
                        cg = part * (pw // CHUNK) + c
                        nc.tensor.matmul(
                            gps[:, bass.ts(c, CHUNK)],
                            w_in[name][:, bass.ts(g, H)],
                            rhs_in[:kin, bass.ts(cg, CHUNK)],
                            start=True, stop=(t == 0))
                    if t > 0:
                        for c in range(pw // CHUNK):
                            cg = part * (pw // CHUNK) + c
                            nc.tensor.matmul(
                                gps[:, bass.ts(c, CHUNK)],
                                w_rec[name][:, bass.ts(g, H)],
                                h_prev[:, bass.ts(cg, CHUNK)],
                                start=False, stop=True)
                    nc.scalar.activation(
                        out=gate[:, part * pw:(part + 1) * pw], in_=gps,
                        func=GATE_FUNCS[g], bias=bias_arg)
                gates[g] = gate
            if t == 0:
                c_tile = cpool.tile([H, SBW], F16, tag=f"c_{name}",
                                    name=f"c_{name}_{t}")
                nc.vector.tensor_tensor(c_tile, gates[0], gates[2], MULT)
            else:
                u = tpool.tile([H, SBW], F16, tag=f"u_{cls}", bufs=2,
                               name=f"u_{name}_{t}")
                nc.vector.tensor_tensor(u, gates[0], gates[2], MULT)
                nc.vector.tensor_tensor(c_tile, gates[1], c_tile, MULT)
                nc.vector.tensor_tensor(c_tile, c_tile, u, ADD)
            return gates[3], c_tile

        def emit_outproj(sb, col0, td, htile):
            ops_t = ps.tile([H, SBW], F32, tag="gps", bufs=2,
                            name=f"op_{sb}_{td}")
            for c in range(N_CHUNKS):
                nc.tensor.matmul(ops_t[:D, bass.ts(c, CHUNK)], w_out,
                                 htile[:, bass.ts(c, CHUNK)],
                                 start=True, stop=True)
            o_t = opool.tile([D, SBW], F32, tag="o", name=f"o_{sb}_{td}")
            nc.vector.tensor_scalar_add(o_t, ops_t[:D, :], b_out)
            nc.sync.dma_start(out=out_ext[td, :, col0:col0 + SBW], in_=o_t)

        def step_tau_h(name, t, o_gate, c_tile, h_tag, h_bufs):
            cls = "A" if name in ("enc0", "dec0") else "B"
            tau = tpool.tile([H, SBW], F16, tag=f"tau_{cls}", bufs=2,
                             name=f"tau_{name}_{t}")
            nc.scalar.activation(out=tau, in_=c_tile, func=TANH)
            h_new = hpool.tile([H, SBW], F16, tag=h_tag, bufs=h_bufs,
                               name=f"h_{name}_{t}")
            nc.vector.tensor_tensor(h_new, o_gate, tau, MULT)
            return h_new

        # tanh(c) as a clamped Pade[5/4] rational on DVE (max err ~2.4e-3 in
        # fp16) for a subset of steps, to offload the saturated ACT engine.
        # Emitted in two pieces at different slot positions so the chain
        # never head-of-line-blocks another stream's DVE cell update.
        def pade_begin(name, t, c_tile):
            cl = tpool.tile([H, SBW], F16, tag=f"cl_{name}", bufs=1,
                            name=f"cl_{name}_{t}")
            nc.vector.tensor_scalar(out=cl, in0=c_tile, scalar1=3.5,
                                    scalar2=-3.5, op0=MIN, op1=MAX)
            u2 = tpool.tile([H, SBW], F16, tag=f"u2_{name}", bufs=1,
                            name=f"u2_{name}_{t}")
            nc.vector.tensor_tensor(u2, cl, cl, MULT)
            qq = tpool.tile([H, SBW], F16, tag=f"qq_{name}", bufs=1,
                            name=f"qq_{name}_{t}")
            nc.vector.tensor_scalar(out=qq, in0=u2, scalar1=1.0 / 63.0,
                                    scalar2=4.0 / 9.0, op0=MULT, op1=ADD)
            nc.vector.tensor_tensor(qq, qq, u2, MULT)
            nc.vector.tensor_scalar(out=qq, in0=qq, scalar1=1.0,
                                    scalar2=None, op0=ADD)
            with nc.allow_low_precision("fp16 pade tanh reciprocal"):
                nc.vector.reciprocal(qq, qq)
            return (name, t, cl, u2, qq)

        def pade_end(state, o_gate, h_tag, h_bufs):
            name, t, cl, u2, qq = state
            pp = tpool.tile([H, SBW], F16, tag=f"pp_{name}", bufs=1,
                            name=f"pp_{name}_{t}")
            nc.vector.tensor_scalar(out=pp, in0=u2, scalar1=1.0 / 945.0,
                                    scalar2=1.0 / 9.0, op0=MULT, op1=ADD)
            nc.vector.tensor_tensor(pp, pp, u2, MULT)
            nc.vector.tensor_scalar(out=pp, in0=pp, scalar1=1.0,
                                    scalar2=None, op0=ADD)
            nc.vector.tensor_tensor(pp, pp, cl, MULT)
            nc.vector.tensor_tensor(cl, pp, qq, MULT)  # cl is dead; reuse
            h_new = hpool.tile([H, SBW], F16, tag=h_tag, bufs=h_bufs,
                               name=f"h_{name}_{t}")
            nc.vector.tensor_tensor(h_new, o_gate, cl, MULT)
            return h_new

        # Software-pipelined over super-batches: while sb k runs its decoder,
        # sb k+1 runs its encoder, so the ACT engine always has a second
        # stream of gate work during latent/out-projection/warm-up phases.
        xq = {}
        x_order = [(s, t) for s in range(N_SB) for t in range(T)]
        x_state = [0]

        def x_ensure(upto_idx):
            while x_state[0] <= min(upto_idx, len(x_order) - 1):
                sbi, ti = x_order[x_state[0]]
                xt = xpool.tile([D + 1, SBW], F16, tag="x",
                                name=f"x_{sbi}_{ti}")
                nc.sync.dma_start(
                    out=xt, in_=x_ext[ti, :, sbi * SBW:(sbi + 1) * SBW])
                xq[(sbi, ti)] = xt
                x_state[0] += 1

        def emit_latent(sb, h1_tile):
            z_t = zpool.tile([L + 1, SBW], F16, tag="z", name=f"z_{sb}")
            lat_ps = ps.tile([H, SBW], F32, tag="gps", bufs=2,
                             name=f"lat_{sb}")
            for c in range(N_CHUNKS):
                nc.tensor.matmul(lat_ps[:L, bass.ts(c, CHUNK)], w_lat,
                                 h1_tile[:, bass.ts(c, CHUNK)],
                                 start=True, stop=True)
            nc.vector.tensor_scalar_add(z_t[:L, :], lat_ps[:L, :], b_lat)
            nc.vector.memset(z_t[L:L + 1, :], 1.0)
            return z_t

        pend_h1 = None
        tw = None
        op_carry = []
        for phase in range(N_SB + 1):
            enc_sb = phase if phase < N_SB else None
            dec_sb = phase - 1 if phase >= 1 else None
            z_dec = None

            if enc_sb is not None:
                ys = [None] * T
                h0 = c0 = h1 = c1 = None
            if dec_sb is not None:
                dcol0 = dec_sb * SBW
                d1 = [None] * T
                hd0 = cd0 = hd1 = cd1 = None
                op_queue = []

            for slot in range(T + 1):
                # Interleave: each layer's tanh/h tail is emitted between the
                # other layers' gate blocks so every h is ready well before
                # the next slot's recurrent matmuls need it, and the ACT
                # engine never drains its queue. The (one-slot-delayed)
                # out-projection sits mid-slot so its DVE-paced PSUM-ring
                # retirement never gates a slot boundary.
                o0 = o1 = od0 = od1 = None
                if enc_sb is not None:
                    if slot < T:
                        x_ensure(enc_sb * T + slot + 2)
                        x_t = xq.pop((enc_sb, slot))
                        o0, c0 = step_gates("enc0", slot, x_t, h0, c0)
                    if slot >= 1:
                        o1, c1 = step_gates("enc1", slot - 1, ys[slot - 1],
                                            h1, c1)
                if tw is None:
                    tw = load_tail_weights()
                    w_lat, b_lat = tw["w_lat"], tw["b_lat"]
                    w_out, b_out = tw["w_out"], tw["b_out"]
                off_e0 = False  # tau offload net-negative under the current
                off_d0 = False  # schedule; pade path kept dormant
                pe0 = None
                if o0 is not None:
                    if off_e0:
                        pe0 = pade_begin("enc0", slot, c0)
                    else:
                        h0 = step_tau_h("enc0", slot, o0, c0, "y0", 3)
                        ys[slot] = h0
                if dec_sb is not None and slot == 0:
                    # latent for this phase's decoder, emitted after the new
                    # encoder's first gates so those cover its latency
                    z_dec = emit_latent(dec_sb, pend_h1)
                pd0 = None
                if dec_sb is not None and slot < T:
                    od0, cd0 = step_gates("dec0", slot, z_dec, hd0, cd0)
                    if off_d0:
                        pd0 = pade_begin("dec0", slot, cd0)
                    else:
                        hd0 = step_tau_h("dec0", slot, od0, cd0, "y1", 3)
                        d1[slot] = hd0
                if o1 is not None:
                    h1 = step_tau_h("enc1", slot - 1, o1, c1, "h1", 2)
                if op_carry:
                    emit_outproj(*op_carry.pop(0))
                if dec_sb is not None and slot >= 1:
                    od1, cd1 = step_gates("dec1", slot - 1, d1[slot - 1],
                                          hd1, cd1)
                if dec_sb is not None and op_queue and (
                        len(op_queue) > 1 or slot == T):
                    emit_outproj(dec_sb, dcol0, *op_queue.pop(0))
                if pe0 is not None:
                    h0 = pade_end(pe0, o0, "y0", 3)
                    ys[slot] = h0
                if pd0 is not None:
                    hd0 = pade_end(pd0, od0, "y1", 3)
                    d1[slot] = hd0
                if od1 is not None:
                    hd1 = step_tau_h("dec1", slot - 1, od1, cd1, "d2", 3)
                    op_queue.append((slot - 1, hd1))

            if dec_sb is not None:
                # leftover out-projections flush in the next phase's early
                # slots so their PSUM-ring retirement never gates the
                # phase boundary
                op_carry = [(dec_sb, dcol0, td, htile)
                            for td, htile in op_queue]
            if enc_sb is not None:
                pend_h1 = h1
        for args in op_carry:
            emit_outproj(*args)

    nc.finalize()
    return nc


def _prep_inputs(inputs):
    """Host-side: transpose/pack fp32 inputs into per-core device arrays."""
    x = inputs["x"]
    xt = np.ascontiguousarray(np.transpose(x, (1, 2, 0)))   # [T, D, B]
    ones = np.ones((T, 1, B), np.float32)
    xt = np.concatenate([xt, ones], axis=1).astype(np.float16)

    common = {}
    for name in LAYERS:
        Wih = inputs[f"{name}_Wih"]
        Whh = inputs[f"{name}_Whh"]
        bsum = (inputs[f"{name}_bih"] + inputs[f"{name}_bhh"]).astype(np.float32)
        w_in = Wih.T.astype(np.float32)                      # [Din, 4H]
        if not LAYERS[name]["act_bias"]:
            w_in = np.concatenate([w_in, bsum[None, :]], axis=0)
        common[f"{name}_w_in"] = w_in.astype(np.float16)
        common[f"{name}_w_rec"] = Whh.T.astype(np.float16)
        if LAYERS[name]["act_bias"]:
            common[f"{name}_bias"] = np.ascontiguousarray(
                bsum.reshape(4, H).T)                        # [H, 4] fp32
    common["w_lat"] = inputs["W_lat"].T.astype(np.float16)   # [H, L]
    common["b_lat"] = inputs["b_lat"].reshape(L, 1).astype(np.float32)
    common["w_out"] = inputs["W_out"].T.astype(np.float16)   # [H, D]
    common["b_out"] = inputs["b_out"].reshape(D, 1).astype(np.float32)

    in_maps = []
    for core in range(N_CORES):
        m = dict(common)
        sl = slice(core * B_CORE, (core + 1) * B_CORE)
        m["x"] = np.ascontiguousarray(xt[:, :, sl])
        in_maps.append(m)
    return in_maps


def bench(inputs, reps: int = 8, reuse_nc=None):
    """Time repeated on-device executions (inputs device-resident, outputs
    left on device). Returns (best_seconds, all_times, outputs_of_first_run).
    """
    import jax
    from jax.sharding import Mesh, NamedSharding, PartitionSpec
    from jax.experimental.shard_map import shard_map
    from concourse import bass2jax
    from concourse.bass2jax import _bass_exec_p, partition_id_tensor

    bass2jax.install_neuronx_cc_hook()
    nc = reuse_nc if reuse_nc is not None else _build_kernel()
    in_maps = _prep_inputs(inputs)
    n_cores = N_CORES

    partition_name = nc.partition_id_tensor.name if nc.partition_id_tensor else None
    in_names, out_names, out_avals, zero_outs = [], [], [], []
    for alloc in nc.m.functions[0].allocations:
        if not isinstance(alloc, mybir.MemoryLocationSet):
            continue
        name = alloc.memorylocations[0].name
        if alloc.kind == "ExternalInput":
            if name != partition_name:
                in_names.append(name)
        elif alloc.kind == "ExternalOutput":
            out_names.append(name)
            out_avals.append(
                jax.core.ShapedArray(tuple(alloc.tensor_shape),
                                     mybir.dt.np(alloc.dtype)))
            zero_outs.append(
                np.zeros(tuple(alloc.tensor_shape), mybir.dt.np(alloc.dtype)))
    n_params = len(in_names)
    n_outs = len(out_names)
    all_in_names = in_names + out_names + ([partition_name] if partition_name else [])
    donate = tuple(range(n_params, n_params + n_outs))

    def _body(*args):
        operands = list(args)
        if partition_name is not None:
            operands.append(partition_id_tensor())
        return tuple(_bass_exec_p.bind(
            *operands, out_avals=tuple(out_avals), in_names=tuple(all_in_names),
            out_names=tuple(out_names), lowering_input_output_aliases=(),
            sim_require_finite=True, sim_require_nnan=True, nc=nc))

    devices = jax.devices()[:n_cores]
    mesh = Mesh(np.asarray(devices), ("core",))
    in_specs = (PartitionSpec("core"),) * (n_params + n_outs)
    out_specs = (PartitionSpec("core"),) * n_outs
    sharded = jax.jit(
        shard_map(_body, mesh=mesh, in_specs=in_specs, out_specs=out_specs,
                  check_rep=False),
        donate_argnums=donate, keep_unused=True)

    shard = NamedSharding(mesh, PartitionSpec("core"))
    concat_in = [
        jax.device_put(
            np.concatenate([np.asarray(in_maps[c][nm]) for c in range(n_cores)], 0),
            shard)
        for nm in in_names
    ]
    def fresh_zeros():
        return [jax.device_put(
                    np.zeros((n_cores * z.shape[0], *z.shape[1:]), z.dtype), shard)
                for z in zero_outs]

    # warm-up (compile)
    outs0 = sharded(*concat_in, *fresh_zeros())
    jax.block_until_ready(outs0)

    zero_sets = [fresh_zeros() for _ in range(reps)]
    jax.block_until_ready(zero_sets)
    times = []
    for r in range(reps):
        t0 = time.perf_counter()
        outs = sharded(*concat_in, *zero_sets[r])
        jax.block_until_ready(outs)
        times.append(time.perf_counter() - t0)
    return min(times), times, outs0


def kernel(**inputs) -> np.ndarray:
    global _last_results
    trace = bool(int(os.environ.get("BASS_LSTM_TRACE", "0")))
    nc = _build_kernel(trace)
    in_maps = _prep_inputs(inputs)
    res = run_bass_kernel_spmd(nc, in_maps, core_ids=list(range(N_CORES)),
                               trace=trace)
    _last_results = res
    outs = [res.results[c]["out"] for c in range(N_CORES)]   # [T, D, B_CORE]
    full = np.concatenate(outs, axis=2)                      # [T, D, B]
    return np.ascontiguousarray(np.transpose(full, (2, 0, 1)))  # [B, T, D]
